# revision 12
# baseline (speedup 1.0000x reference)
"""Trainium2 Bass kernel for fused causal GQA attention block.

Reference computation (B=1, S=2048, H=4096, NH=32, NKV=8, HD=128):
    qkv = hs @ w_attn.T; rope(q), rope(k); causal GQA attention;
    out @ w_proj.T

Sharding (8 cores, tensor parallel): core i owns kv-group i = rows
[i*768, (i+1)*768) of w_attn (4 q heads + 1 k + 1 v head) and rows
[i*512, (i+1)*512) of w_proj.  Each core computes its 4 heads of
attention output transposed (feature-major); a seq-chunked AllGather
assembles attnT on every core, and each core computes its 512 output
columns of the final projection per seq chunk.

v2 design (vs the f32r baseline):
  * all matmul/DMA data in bf16 (host-converted) -- halves HBM and
    collective traffic and the LDWEIGHTS stream; psums stay fp32.
  * RoPE + v-transposes interleaved into phase A per 512-seq block so
    attention starts immediately after the last qkv block.
  * attention runs seq chunks in order iq=3,2,1,0 (largest first) with
    each chunk's AllGather issued right away and its c_proj placed one
    chunk later, so every collective hides under compute.
  * causal mask applied on the PE as a staircase bias-matmul into the
    scores psum (lhsT=identity) -- no DVE hop between exp and l/o.
  * scores software-pipelined 3 deep over key tiles; exp on ACT.
  * softmax reciprocal via reciprocal_approx_fast (~5x faster).
"""

import sys

sys.path.insert(0, "/opt/trn_rl_repo")

import ml_dtypes
import numpy as np

import concourse.bass as bass
import concourse.tile as tile
from concourse import bacc, mybir
from concourse.bass_utils import run_bass_kernel_spmd

F32 = mybir.dt.float32
BF16 = mybir.dt.bfloat16

B, S, H = 1, 2048, 4096
NH, NKV, HD = 32, 8, 128
GROUP = NH // NKV  # 4
SCALE = 0.08838834764831845
NCORES = 8

M_SHARD = (GROUP + 2) * HD  # 768 rows of w_attn per core
P_SHARD = H // NCORES  # 512 rows of w_proj per core

KC = H // 128  # 32 contraction chunks of the model dim
NB = S // 512  # 4 seq blocks of 512
MT = M_SHARD // 128  # 6 row tiles of qkv_t
ST = S // 128  # 16 seq tiles of 128
NEG = -1.0e9


def build_module() -> bass.Bass:
    nc = bacc.Bacc(
        "TRN2",
        target_bir_lowering=False,
        debug=False,
        num_devices=NCORES,
    )

    hs_t = nc.dram_tensor("hs_t", [H, S], BF16, kind="ExternalInput")
    wa_t = nc.dram_tensor("wa_t", [H, M_SHARD], BF16, kind="ExternalInput")
    wp_t = nc.dram_tensor("wp_t", [H, P_SHARD], BF16, kind="ExternalInput")
    cos_t = nc.dram_tensor("cos_t", [HD, S], BF16, kind="ExternalInput")
    sin_t = nc.dram_tensor("sin_t", [HD, S], BF16, kind="ExternalInput")
    rot_t = nc.dram_tensor("rot_t", [HD, HD], BF16, kind="ExternalInput")
    stair_in = nc.dram_tensor("stair_in", [128, 512], BF16, kind="ExternalInput")
    ones_in = nc.dram_tensor("ones_in", [128, 128], BF16, kind="ExternalInput")
    ident_in = nc.dram_tensor("ident_in", [128, 128], BF16, kind="ExternalInput")
    y_out = nc.dram_tensor("y", [S, P_SHARD], F32, kind="ExternalOutput")

    # per-seq-chunk collective buffers
    ag_ins = [
        nc.dram_tensor(f"ag_in{i}", [GROUP * HD, 512], BF16, kind="Internal")
        for i in range(NB)
    ]
    ag_outs = [
        nc.dram_tensor(
            f"ag_out{i}", [H, 512], BF16, kind="Internal", addr_space="Shared"
        )
        for i in range(NB)
    ]

    # DRAM views with 128-partition tiling of the contraction axis
    hs_v = hs_t[:].rearrange("(ko p) n -> p ko n", p=128)  # [128, 32, 2048]
    wa_v = wa_t[:].rearrange("(ko p) m -> p ko m", p=128)  # [128, 32, 768]
    wp_v = wp_t[:].rearrange("(ko p) m -> p ko m", p=128)  # [128, 32, 512]
    ag_vs = [a[:].rearrange("(ko p) n -> p ko n", p=128) for a in ag_outs]

    with tile.TileContext(nc) as tc:
        # ---------- persistent pools ----------
        const_pool = tc.alloc_tile_pool(name="consts", bufs=1)
        qkv_pool = tc.alloc_tile_pool(name="qkv", bufs=1)
        vnat_pool = tc.alloc_tile_pool(name="vnat", bufs=1)
        wp_pool = tc.alloc_tile_pool(name="wp", bufs=1)

        ones_sb = const_pool.tile([128, 128], BF16)
        ident_sb = const_pool.tile([128, 128], BF16)
        rot_sb = const_pool.tile([128, HD], BF16)
        stair_sb = const_pool.tile([128, 512], BF16)

        qkv_sb = qkv_pool.tile([128, MT, S], BF16)  # 24KB/part
        v_nat = vnat_pool.tile([128, ST, HD], BF16)  # 4KB/part
        wp_sb = wp_pool.tile([128, KC, P_SHARD], BF16)  # 32KB/part

        # ---------- phase A: qkv_t = wa_shard @ hs.T, rope fused ----------
        with (
            tc.tile_pool(name="wa", bufs=1) as wa_pool,
            tc.tile_pool(name="hs", bufs=2) as hs_pool,
            tc.tile_pool(name="rope", bufs=1) as rope_pool,
            tc.tile_pool(name="rs", bufs=2) as rs_pool,
            tc.tile_pool(name="psA", bufs=1, space="PSUM") as psA,
            tc.tile_pool(name="psR", bufs=2, space="PSUM") as psR,
        ):
            wa_sb = wa_pool.tile([128, KC, M_SHARD], BF16)  # 48KB/part
            cos_sb = rope_pool.tile([128, S], BF16, tag="cos")
            sin_sb = rope_pool.tile([128, S], BF16, tag="sin")
            rope_tables_loaded = False

            # wa chunks first on scalar (PE needs chunk0 immediately); the
            # first hs slab is first on sync.  Constants / rope tables /
            # w_proj stream behind — they're needed much later.
            for kk in range(0, KC, 8):
                nc.scalar.dma_start(
                    out=wa_sb[:, kk : kk + 8, :], in_=wa_v[:, kk : kk + 8, :]
                )
            nc.scalar.dma_start(out=rot_sb, in_=rot_t[:])
            nc.scalar.dma_start(out=ones_sb, in_=ones_in[:])
            nc.scalar.dma_start(out=ident_sb, in_=ident_in[:])
            nc.scalar.dma_start(out=stair_sb, in_=stair_in[:])
            for kk in range(0, KC, 8):
                nc.scalar.dma_start(
                    out=wp_sb[:, kk : kk + 8, :], in_=wp_v[:, kk : kk + 8, :]
                )

            KSLAB = 8
            for nb in range(NB):
                blk = slice(nb * 512, (nb + 1) * 512)
                psums = [
                    psA.tile([128, 512], F32, tag=f"ps{m}", name=f"psA{m}")
                    for m in range(MT)
                ]
                for ks in range(0, KC, KSLAB):
                    hs_slab = hs_pool.tile([128, KSLAB, 512], BF16, name="hs_slab")
                    nc.sync.dma_start(out=hs_slab, in_=hs_v[:, ks : ks + KSLAB, blk])
                    if not rope_tables_loaded:
                        # behind the first slab; needed only at nb0's rope
                        nc.sync.dma_start(out=cos_sb, in_=cos_t[:])
                        nc.sync.dma_start(out=sin_sb, in_=sin_t[:])
                        rope_tables_loaded = True
                    for k in range(ks, ks + KSLAB):
                        for m in range(MT):
                            nc.tensor.matmul(
                                psums[m],
                                lhsT=wa_sb[:, k, m * 128 : (m + 1) * 128],
                                rhs=hs_slab[:, k - ks, :],
                                start=(k == 0),
                                stop=(k == KC - 1),
                            )
                for m in range(MT):
                    nc.vector.tensor_copy(out=qkv_sb[:, m, blk], in_=psums[m])

                # v natural layout via DMA-transpose (sync queue, tiny)
                for u in range(4):
                    nc.sync.dma_start_transpose(
                        v_nat[:, nb * 4 + u, :],
                        qkv_sb[:, GROUP + 1, nb * 512 + u * 128 : nb * 512 + (u + 1) * 128],
                    )

                # rope this seq block, k tile first (attention needs kT whole)
                for t in (GROUP, 0, 1, 2, 3):
                    x = qkv_sb[:, t, blk]
                    rp = psR.tile([128, 512], F32, name="rp")
                    nc.tensor.matmul(rp, lhsT=rot_sb[:], rhs=x, start=True, stop=True)
                    rs = rs_pool.tile([128, 512], BF16, name="rs")
                    nc.vector.tensor_mul(rs, rp, sin_sb[:, blk])
                    nc.vector.tensor_mul(x, x, cos_sb[:, blk])
                    nc.vector.tensor_add(x, x, rs)

        # ---------- phase B: attention + chunked AG + c_proj ----------
        with (
            tc.tile_pool(name="pt", bufs=1) as pt_pool,
            tc.tile_pool(name="attn", bufs=2) as attn_pool,
            tc.tile_pool(name="lh", bufs=4) as lh_pool,
            tc.tile_pool(name="ysb", bufs=2) as y_pool,
            tc.tile_pool(name="psS", bufs=3, space="PSUM") as psS,
            tc.tile_pool(name="psL", bufs=1, space="PSUM") as psL,
            tc.tile_pool(name="psO", bufs=2, space="PSUM") as psO,
            tc.tile_pool(name="psC", bufs=2, space="PSUM") as psC,
        ):
            # pre-zero the shifted-diagonal pt tags' dead zones once
            for r in (128, 256, 384):
                ptd = pt_pool.tile(
                    [128, 512], BF16, tag=f"ptd{r}", name=f"ptd{r}"
                )
                nc.vector.memset(ptd[:, 0:r], 0.0)

            kT = qkv_sb[:, GROUP, :]
            lh_pending = {}  # iq -> list of (sub, lh tile)

            def emit_lh(iq, sub):
                # sync queue: the scalar queue carries the exp stream, which
                # must never sit behind a DMA that waits on an AllGather
                lh = lh_pool.tile([128, KC, 128], BF16, tag="lh", name="lh")
                nc.sync.dma_start(
                    out=lh, in_=ag_vs[iq][:, :, sub * 128 : (sub + 1) * 128]
                )
                return lh

            def cproj(iq):
                tiles = lh_pending.pop(iq)
                while len(tiles) < 4:
                    tiles.append(emit_lh(iq, len(tiles)))
                for sub in range(4):
                    lh = tiles[sub]
                    yp = psC.tile([128, 512], F32, name="yp")
                    for k in range(KC):
                        nc.tensor.matmul(
                            yp,
                            lhsT=lh[:, k, :],
                            rhs=wp_sb[:, k, :],
                            start=(k == 0),
                            stop=(k == KC - 1),
                        )
                    ysb = y_pool.tile([128, P_SHARD], F32, name="ysb")
                    nc.vector.tensor_copy(out=ysb, in_=yp)
                    nc.sync.dma_start(
                        out=y_out[(iq * 4 + sub) * 128 : (iq * 4 + sub + 1) * 128, :],
                        in_=ysb,
                    )

            order = [3, 2, 1, 0]
            for idx, iq in enumerate(order):
                njb = 4 * iq + 4
                q0 = iq * 512
                for h in range(GROUP):
                    l_ps = psL.tile([128, 512], F32, name="l_ps")
                    o_ps = psO.tile([128, 512], F32, name="o_ps")

                    # round r: key tile j=r; last 4 rounds are diagonal with
                    # shifted q-range [rel, 512) and a staircase bias matmul
                    sts = []  # per round: (st psum, pt tile, rel)

                    def emit_st(r):
                        j = r
                        rel = max(0, (j - 4 * iq) * 128)
                        st = psS.tile([128, 512], F32, name="st")
                        nc.tensor.matmul(
                            st[:, rel:512],
                            lhsT=kT[:, j * 128 : (j + 1) * 128],
                            rhs=qkv_sb[:, h, q0 + rel : q0 + 512],
                            start=True,
                            stop=(rel == 0),
                        )
                        if rel > 0:
                            nc.tensor.matmul(
                                st[:, rel:512],
                                lhsT=ident_sb[:],
                                rhs=stair_sb[:, 0 : 512 - rel],
                                start=False,
                                stop=True,
                            )
                        tag = f"ptd{rel}" if rel else "pt"
                        bufs = 1 if rel else 4
                        pt = pt_pool.tile(
                            [128, 512], BF16, tag=tag, bufs=bufs, name="pt"
                        )
                        nc.scalar.activation(
                            out=pt[:, rel:512],
                            in_=st[:, rel:512],
                            func=mybir.ActivationFunctionType.Exp,
                            scale=SCALE,
                        )
                        sts.append((st, pt, rel))

                    def emit_st_diag0(r):
                        # diagonal tile at rel==0 (j == 4*iq): triangle bias
                        j = r
                        st = psS.tile([128, 512], F32, name="st")
                        nc.tensor.matmul(
                            st,
                            lhsT=kT[:, j * 128 : (j + 1) * 128],
                            rhs=qkv_sb[:, h, q0 : q0 + 512],
                            start=True,
                            stop=False,
                        )
                        nc.tensor.matmul(
                            st,
                            lhsT=ident_sb[:],
                            rhs=stair_sb[:],
                            start=False,
                            stop=True,
                        )
                        pt = pt_pool.tile([128, 512], BF16, tag="pt", bufs=4, name="pt")
                        nc.scalar.activation(
                            out=pt,
                            in_=st,
                            func=mybir.ActivationFunctionType.Exp,
                            scale=SCALE,
                        )
                        sts.append((st, pt, 0))

                    def emit_round_st(r):
                        if r == 4 * iq:
                            emit_st_diag0(r)
                        else:
                            emit_st(r)

                    def emit_lo(r):
                        _, pt, _ = sts[r]
                        j = r
                        nc.tensor.matmul(
                            l_ps,
                            lhsT=ones_sb[:],
                            rhs=pt[:],
                            start=(r == 0),
                            stop=(r == njb - 1),
                        )
                        nc.tensor.matmul(
                            o_ps,
                            lhsT=v_nat[:, j, :],
                            rhs=pt[:],
                            start=(r == 0),
                            stop=(r == njb - 1),
                        )

                    depth = min(3, njb)
                    for r in range(depth):
                        emit_round_st(r)
                    for r in range(njb):
                        emit_lo(r)
                        if r + depth < njb:
                            emit_round_st(r + depth)

                    linv = attn_pool.tile([128, 512], F32, tag="linv", name="linv")
                    nc.vector.reciprocal_approx_fast(out=linv, in_=l_ps)
                    at = attn_pool.tile([128, 512], BF16, tag="at", name="at")
                    nc.vector.tensor_mul(at, o_ps, linv)
                    nc.sync.dma_start(
                        out=ag_ins[iq][h * 128 : (h + 1) * 128, :], in_=at
                    )

                # seq-chunked AllGather; overlaps the next chunk's compute
                nc.gpsimd.collective_compute(
                    "AllGather",
                    mybir.AluOpType.bypass,
                    replica_groups=[list(range(NCORES))],
                    ins=[ag_ins[iq][:]],
                    outs=[ag_outs[iq][:]],
                )

                if idx >= 1:
                    cproj(order[idx - 1])
                # prefetch first half of this chunk's c_proj activations
                # (after the interleaved cproj so its lh DMAs, which wait on
                # this chunk's AllGather, don't block the queue)
                lh_pending[iq] = [emit_lh(iq, 0), emit_lh(iq, 1)]
            cproj(order[-1])

        wp_pool.release()
        vnat_pool.release()
        qkv_pool.release()
        const_pool.release()

    nc.compile()
    return nc


_CACHED = {}


def _get_module():
    if "nc" not in _CACHED:
        _CACHED["nc"] = build_module()
    return _CACHED["nc"]


def make_in_maps(hidden_states, w_attn, w_proj, rope_cos, rope_sin):
    bf = ml_dtypes.bfloat16
    hidden_states = np.asarray(hidden_states, dtype=np.float32)
    w_attn = np.asarray(w_attn, dtype=np.float32)
    w_proj = np.asarray(w_proj, dtype=np.float32)
    rope_cos = np.asarray(rope_cos, dtype=np.float32)
    rope_sin = np.asarray(rope_sin, dtype=np.float32)

    hs_t = np.ascontiguousarray(hidden_states.reshape(S, H).T).astype(bf)
    cos_t = np.ascontiguousarray(rope_cos.T).astype(bf)
    sin_t = np.ascontiguousarray(rope_sin.T).astype(bf)

    # rotate-half as a matmul: rot(x) = R @ x for x in [HD, S] layout,
    # rot_t = R.T so that lhsT.T @ x = R @ x
    rot_t = np.zeros((HD, HD), dtype=np.float32)
    half = HD // 2
    rot_t[half + np.arange(half), np.arange(half)] = -1.0
    rot_t[np.arange(half), half + np.arange(half)] = 1.0

    # causal triangle bias: stair[k, u] = NEG iff u < k (else 0); a diag
    # key tile at relative offset rel uses cols [0, 512-rel)
    kk, uu = np.meshgrid(np.arange(128), np.arange(512), indexing="ij")
    stair = np.where(uu < kk, NEG, 0.0).astype(np.float32)

    ones = np.ones((128, 128), dtype=np.float32)
    ident = np.eye(128, dtype=np.float32)

    in_maps = []
    for i in range(NCORES):
        wa_sh = w_attn[i * M_SHARD : (i + 1) * M_SHARD, :]
        wp_sh = w_proj[i * P_SHARD : (i + 1) * P_SHARD, :]
        in_maps.append(
            {
                "hs_t": hs_t,
                "wa_t": np.ascontiguousarray(wa_sh.T).astype(bf),
                "wp_t": np.ascontiguousarray(wp_sh.T).astype(bf),
                "cos_t": cos_t,
                "sin_t": sin_t,
                "rot_t": rot_t.astype(bf),
                "stair_in": stair.astype(bf),
                "ones_in": ones.astype(bf),
                "ident_in": ident.astype(bf),
            }
        )
    return in_maps


def kernel(hidden_states, w_attn, w_proj, rope_cos, rope_sin, **_unused):
    nc = _get_module()
    in_maps = make_in_maps(hidden_states, w_attn, w_proj, rope_cos, rope_sin)
    res = run_bass_kernel_spmd(nc, in_maps, core_ids=list(range(NCORES)))

    out = np.empty((S, H), dtype=np.float32)
    for i in range(NCORES):
        out[:, i * P_SHARD : (i + 1) * P_SHARD] = res.results[i]["y"]
    return out.reshape(B, S, H)


# revision 14
# speedup vs baseline: 1.1209x; 1.1209x over previous
"""Trainium2 Bass kernel for fused causal GQA attention block.

Reference computation (B=1, S=2048, H=4096, NH=32, NKV=8, HD=128):
    qkv = hs @ w_attn.T; rope(q), rope(k); causal GQA attention;
    out @ w_proj.T

Sharding (8 cores, tensor parallel): core i owns kv-group i = rows
[i*768, (i+1)*768) of w_attn (4 q heads + 1 k + 1 v head) and rows
[i*512, (i+1)*512) of w_proj.  Each core computes its 4 heads of
attention output transposed (feature-major); a seq-chunked AllGather
assembles attnT on every core, and each core computes its 512 output
columns of the final projection per seq chunk.

v2 design (vs the f32r baseline):
  * all matmul/DMA data in bf16 (host-converted) -- halves HBM and
    collective traffic and the LDWEIGHTS stream; psums stay fp32.
  * RoPE + v-transposes interleaved into phase A per 512-seq block so
    attention starts immediately after the last qkv block.
  * attention runs seq chunks in order iq=3,2,1,0 (largest first) with
    each chunk's AllGather issued right away and its c_proj placed one
    chunk later, so every collective hides under compute.
  * causal mask applied on the PE as a staircase bias-matmul into the
    scores psum (lhsT=identity) -- no DVE hop between exp and l/o.
  * scores software-pipelined 3 deep over key tiles; exp on ACT.
  * softmax reciprocal via reciprocal_approx_fast (~5x faster).
"""

import sys

sys.path.insert(0, "/opt/trn_rl_repo")

import ml_dtypes
import numpy as np

import concourse.bass as bass
import concourse.tile as tile
from concourse import bacc, mybir
from concourse.bass_utils import run_bass_kernel_spmd

F32 = mybir.dt.float32
BF16 = mybir.dt.bfloat16

B, S, H = 1, 2048, 4096
NH, NKV, HD = 32, 8, 128
GROUP = NH // NKV  # 4
SCALE = 0.08838834764831845
NCORES = 8

M_SHARD = (GROUP + 2) * HD  # 768 rows of w_attn per core
P_SHARD = H // NCORES  # 512 rows of w_proj per core

KC = H // 128  # 32 contraction chunks of the model dim
NB = S // 512  # 4 seq blocks of 512
MT = M_SHARD // 128  # 6 row tiles of qkv_t
ST = S // 128  # 16 seq tiles of 128
NEG = -1.0e9


def build_module() -> bass.Bass:
    nc = bacc.Bacc(
        "TRN2",
        target_bir_lowering=False,
        debug=False,
        num_devices=NCORES,
    )

    hs_t = nc.dram_tensor("hs_t", [H, S], BF16, kind="ExternalInput")
    wa_t = nc.dram_tensor("wa_t", [H, M_SHARD], BF16, kind="ExternalInput")
    wp_t = nc.dram_tensor("wp_t", [H, P_SHARD], BF16, kind="ExternalInput")
    cos_t = nc.dram_tensor("cos_t", [HD, S], BF16, kind="ExternalInput")
    sin_t = nc.dram_tensor("sin_t", [HD, S], BF16, kind="ExternalInput")
    rot_t = nc.dram_tensor("rot_t", [HD, HD], BF16, kind="ExternalInput")
    stair_in = nc.dram_tensor("stair_in", [128, 512], BF16, kind="ExternalInput")
    ones_in = nc.dram_tensor("ones_in", [128, 128], BF16, kind="ExternalInput")
    ident_in = nc.dram_tensor("ident_in", [128, 128], BF16, kind="ExternalInput")
    y_out = nc.dram_tensor("y", [S, P_SHARD], F32, kind="ExternalOutput")

    # per-seq-chunk collective buffers
    ag_ins = [
        nc.dram_tensor(f"ag_in{i}", [GROUP * HD, 512], BF16, kind="Internal")
        for i in range(NB)
    ]
    ag_outs = [
        nc.dram_tensor(
            f"ag_out{i}", [H, 512], BF16, kind="Internal", addr_space="Shared"
        )
        for i in range(NB)
    ]

    # DRAM views with 128-partition tiling of the contraction axis
    hs_v = hs_t[:].rearrange("(ko p) n -> p ko n", p=128)  # [128, 32, 2048]
    wa_v = wa_t[:].rearrange("(ko p) m -> p ko m", p=128)  # [128, 32, 768]
    wp_v = wp_t[:].rearrange("(ko p) m -> p ko m", p=128)  # [128, 32, 512]
    ag_vs = [a[:].rearrange("(ko p) n -> p ko n", p=128) for a in ag_outs]

    with tile.TileContext(nc) as tc:
        # ---------- persistent pools ----------
        const_pool = tc.alloc_tile_pool(name="consts", bufs=1)
        qkv_pool = tc.alloc_tile_pool(name="qkv", bufs=1)
        vnat_pool = tc.alloc_tile_pool(name="vnat", bufs=1)
        wp_pool = tc.alloc_tile_pool(name="wp", bufs=1)

        ones_sb = const_pool.tile([128, 128], BF16)
        ident_sb = const_pool.tile([128, 128], BF16)
        rot_sb = const_pool.tile([128, HD], BF16)
        stair_sb = const_pool.tile([128, 512], BF16)

        qkv_sb = qkv_pool.tile([128, MT, S], BF16)  # 24KB/part
        v_nat = vnat_pool.tile([128, ST, HD], BF16)  # 4KB/part
        wp_sb = wp_pool.tile([128, KC, P_SHARD], BF16)  # 32KB/part

        # ---------- phase A: qkv_t = wa_shard @ hs.T, rope fused ----------
        with (
            tc.tile_pool(name="wa", bufs=1) as wa_pool,
            tc.tile_pool(name="hs", bufs=2) as hs_pool,
            tc.tile_pool(name="rope", bufs=1) as rope_pool,
            tc.tile_pool(name="rs", bufs=2) as rs_pool,
            tc.tile_pool(name="psA", bufs=1, space="PSUM") as psA,
            tc.tile_pool(name="psR", bufs=2, space="PSUM") as psR,
        ):
            wa_sb = wa_pool.tile([128, KC, M_SHARD], BF16)  # 48KB/part
            cos_sb = rope_pool.tile([128, S], BF16, tag="cos")
            sin_sb = rope_pool.tile([128, S], BF16, tag="sin")
            rope_tables_loaded = False

            # wa chunks first on scalar (PE needs chunk0 immediately); the
            # first hs slab is first on sync.  Constants / rope tables /
            # w_proj stream behind — they're needed much later.
            for kk in range(0, KC, 8):
                nc.scalar.dma_start(
                    out=wa_sb[:, kk : kk + 8, :], in_=wa_v[:, kk : kk + 8, :]
                )
            nc.scalar.dma_start(out=rot_sb, in_=rot_t[:])
            nc.scalar.dma_start(out=ones_sb, in_=ones_in[:])
            nc.scalar.dma_start(out=ident_sb, in_=ident_in[:])
            nc.scalar.dma_start(out=stair_sb, in_=stair_in[:])
            for kk in range(0, KC, 8):
                nc.scalar.dma_start(
                    out=wp_sb[:, kk : kk + 8, :], in_=wp_v[:, kk : kk + 8, :]
                )

            KSLAB = 8
            for nb in range(NB):
                blk = slice(nb * 512, (nb + 1) * 512)
                psums = [
                    psA.tile([128, 512], F32, tag=f"ps{m}", name=f"psA{m}")
                    for m in range(MT)
                ]
                for ks in range(0, KC, KSLAB):
                    hs_slab = hs_pool.tile([128, KSLAB, 512], BF16, name="hs_slab")
                    nc.sync.dma_start(out=hs_slab, in_=hs_v[:, ks : ks + KSLAB, blk])
                    if not rope_tables_loaded:
                        # behind the first slab; needed only at nb0's rope
                        nc.sync.dma_start(out=cos_sb, in_=cos_t[:])
                        nc.sync.dma_start(out=sin_sb, in_=sin_t[:])
                        rope_tables_loaded = True
                    for k in range(ks, ks + KSLAB):
                        for m in range(MT):
                            nc.tensor.matmul(
                                psums[m],
                                lhsT=wa_sb[:, k, m * 128 : (m + 1) * 128],
                                rhs=hs_slab[:, k - ks, :],
                                start=(k == 0),
                                stop=(k == KC - 1),
                            )
                for m in range(MT):
                    nc.vector.tensor_copy(out=qkv_sb[:, m, blk], in_=psums[m])

                # v natural layout via DMA-transpose (sync queue, tiny)
                for u in range(4):
                    nc.sync.dma_start_transpose(
                        v_nat[:, nb * 4 + u, :],
                        qkv_sb[:, GROUP + 1, nb * 512 + u * 128 : nb * 512 + (u + 1) * 128],
                    )

                # rope this seq block, k tile first (attention needs kT whole)
                for t in (GROUP, 0, 1, 2, 3):
                    x = qkv_sb[:, t, blk]
                    rp = psR.tile([128, 512], F32, name="rp")
                    nc.tensor.matmul(rp, lhsT=rot_sb[:], rhs=x, start=True, stop=True)
                    rs = rs_pool.tile([128, 512], BF16, name="rs")
                    nc.vector.tensor_mul(rs, rp, sin_sb[:, blk])
                    nc.vector.tensor_mul(x, x, cos_sb[:, blk])
                    nc.vector.tensor_add(x, x, rs)

        # ---------- phase B: attention + chunked AG + c_proj ----------
        with (
            tc.tile_pool(name="pt", bufs=1) as pt_pool,
            tc.tile_pool(name="attn", bufs=2) as attn_pool,
            tc.tile_pool(name="lh", bufs=4) as lh_pool,
            tc.tile_pool(name="ysb", bufs=2) as y_pool,
            tc.tile_pool(name="psS", bufs=3, space="PSUM") as psS,
            tc.tile_pool(name="psL", bufs=1, space="PSUM") as psL,
            tc.tile_pool(name="psO", bufs=2, space="PSUM") as psO,
            tc.tile_pool(name="psC", bufs=2, space="PSUM") as psC,
        ):
            # pre-zero the shifted-diagonal pt tags' dead zones once
            for r in (128, 256, 384):
                ptd = pt_pool.tile(
                    [128, 512], BF16, tag=f"ptd{r}", name=f"ptd{r}"
                )
                nc.vector.memset(ptd[:, 0:r], 0.0)

            kT = qkv_sb[:, GROUP, :]

            def emit_lh(iq, sub):
                # scalar queue, emitted at cproj start: by then this chunk's
                # AllGather has long completed, so the trigger never parks at
                # the queue head (a sync-queue DMA blocked on a collective
                # stalls the CC stream itself -- measured 96us).
                lh = lh_pool.tile([128, KC, 128], BF16, tag="lh", name="lh")
                nc.scalar.dma_start(
                    out=lh, in_=ag_vs[iq][:, :, sub * 128 : (sub + 1) * 128]
                )
                return lh

            def cproj(iq):
                tiles = [emit_lh(iq, sub) for sub in range(4)]
                for sub in range(4):
                    lh = tiles[sub]
                    yp = psC.tile([128, 512], F32, name="yp")
                    for k in range(KC):
                        nc.tensor.matmul(
                            yp,
                            lhsT=lh[:, k, :],
                            rhs=wp_sb[:, k, :],
                            start=(k == 0),
                            stop=(k == KC - 1),
                        )
                    ysb = y_pool.tile([128, P_SHARD], F32, name="ysb")
                    nc.vector.tensor_copy(out=ysb, in_=yp)
                    nc.sync.dma_start(
                        out=y_out[(iq * 4 + sub) * 128 : (iq * 4 + sub + 1) * 128, :],
                        in_=ysb,
                    )

            order = [3, 2, 1, 0]
            for idx, iq in enumerate(order):
                njb = 4 * iq + 4
                q0 = iq * 512
                for h in range(GROUP):
                    l_ps = psL.tile([128, 512], F32, name="l_ps")
                    o_ps = psO.tile([128, 512], F32, name="o_ps")

                    # round r: key tile j=r; last 4 rounds are diagonal with
                    # shifted q-range [rel, 512) and a staircase bias matmul
                    sts = []  # per round: (st psum, pt tile, rel)

                    def emit_st(r):
                        j = r
                        rel = max(0, (j - 4 * iq) * 128)
                        st = psS.tile([128, 512], F32, name="st")
                        nc.tensor.matmul(
                            st[:, rel:512],
                            lhsT=kT[:, j * 128 : (j + 1) * 128],
                            rhs=qkv_sb[:, h, q0 + rel : q0 + 512],
                            start=True,
                            stop=(rel == 0),
                        )
                        if rel > 0:
                            nc.tensor.matmul(
                                st[:, rel:512],
                                lhsT=ident_sb[:],
                                rhs=stair_sb[:, 0 : 512 - rel],
                                start=False,
                                stop=True,
                            )
                        tag = f"ptd{rel}" if rel else "pt"
                        bufs = 1 if rel else 4
                        pt = pt_pool.tile(
                            [128, 512], BF16, tag=tag, bufs=bufs, name="pt"
                        )
                        nc.scalar.activation(
                            out=pt[:, rel:512],
                            in_=st[:, rel:512],
                            func=mybir.ActivationFunctionType.Exp,
                            scale=SCALE,
                        )
                        sts.append((st, pt, rel))

                    def emit_st_diag0(r):
                        # diagonal tile at rel==0 (j == 4*iq): triangle bias
                        j = r
                        st = psS.tile([128, 512], F32, name="st")
                        nc.tensor.matmul(
                            st,
                            lhsT=kT[:, j * 128 : (j + 1) * 128],
                            rhs=qkv_sb[:, h, q0 : q0 + 512],
                            start=True,
                            stop=False,
                        )
                        nc.tensor.matmul(
                            st,
                            lhsT=ident_sb[:],
                            rhs=stair_sb[:],
                            start=False,
                            stop=True,
                        )
                        pt = pt_pool.tile([128, 512], BF16, tag="pt", bufs=4, name="pt")
                        nc.scalar.activation(
                            out=pt,
                            in_=st,
                            func=mybir.ActivationFunctionType.Exp,
                            scale=SCALE,
                        )
                        sts.append((st, pt, 0))

                    def emit_round_st(r):
                        if r == 4 * iq:
                            emit_st_diag0(r)
                        else:
                            emit_st(r)

                    def emit_lo(r):
                        _, pt, _ = sts[r]
                        j = r
                        nc.tensor.matmul(
                            l_ps,
                            lhsT=ones_sb[:],
                            rhs=pt[:],
                            start=(r == 0),
                            stop=(r == njb - 1),
                        )
                        nc.tensor.matmul(
                            o_ps,
                            lhsT=v_nat[:, j, :],
                            rhs=pt[:],
                            start=(r == 0),
                            stop=(r == njb - 1),
                        )

                    depth = min(3, njb)
                    for r in range(depth):
                        emit_round_st(r)
                    for r in range(njb):
                        emit_lo(r)
                        if r + depth < njb:
                            emit_round_st(r + depth)

                    linv = attn_pool.tile([128, 512], F32, tag="linv", name="linv")
                    nc.vector.reciprocal_approx_fast(out=linv, in_=l_ps)
                    at = attn_pool.tile([128, 512], BF16, tag="at", name="at")
                    nc.vector.tensor_mul(at, o_ps, linv)
                    nc.sync.dma_start(
                        out=ag_ins[iq][h * 128 : (h + 1) * 128, :], in_=at
                    )

                # seq-chunked AllGather; overlaps the next chunk's compute
                nc.gpsimd.collective_compute(
                    "AllGather",
                    mybir.AluOpType.bypass,
                    replica_groups=[list(range(NCORES))],
                    ins=[ag_ins[iq][:]],
                    outs=[ag_outs[iq][:]],
                )

                # two-chunk-delayed c_proj: chunk iq's AllGather gets two
                # attention chunks (~60us) to complete before its consumer
                if idx >= 2:
                    cproj(order[idx - 2])
            cproj(order[-2])
            cproj(order[-1])

        wp_pool.release()
        vnat_pool.release()
        qkv_pool.release()
        const_pool.release()

    nc.compile()
    return nc


_CACHED = {}


def _get_module():
    if "nc" not in _CACHED:
        _CACHED["nc"] = build_module()
    return _CACHED["nc"]


def make_in_maps(hidden_states, w_attn, w_proj, rope_cos, rope_sin):
    bf = ml_dtypes.bfloat16
    hidden_states = np.asarray(hidden_states, dtype=np.float32)
    w_attn = np.asarray(w_attn, dtype=np.float32)
    w_proj = np.asarray(w_proj, dtype=np.float32)
    rope_cos = np.asarray(rope_cos, dtype=np.float32)
    rope_sin = np.asarray(rope_sin, dtype=np.float32)

    hs_t = np.ascontiguousarray(hidden_states.reshape(S, H).T).astype(bf)
    cos_t = np.ascontiguousarray(rope_cos.T).astype(bf)
    sin_t = np.ascontiguousarray(rope_sin.T).astype(bf)

    # rotate-half as a matmul: rot(x) = R @ x for x in [HD, S] layout,
    # rot_t = R.T so that lhsT.T @ x = R @ x
    rot_t = np.zeros((HD, HD), dtype=np.float32)
    half = HD // 2
    rot_t[half + np.arange(half), np.arange(half)] = -1.0
    rot_t[np.arange(half), half + np.arange(half)] = 1.0

    # causal triangle bias: stair[k, u] = NEG iff u < k (else 0); a diag
    # key tile at relative offset rel uses cols [0, 512-rel)
    kk, uu = np.meshgrid(np.arange(128), np.arange(512), indexing="ij")
    stair = np.where(uu < kk, NEG, 0.0).astype(np.float32)

    ones = np.ones((128, 128), dtype=np.float32)
    ident = np.eye(128, dtype=np.float32)

    in_maps = []
    for i in range(NCORES):
        wa_sh = w_attn[i * M_SHARD : (i + 1) * M_SHARD, :]
        wp_sh = w_proj[i * P_SHARD : (i + 1) * P_SHARD, :]
        in_maps.append(
            {
                "hs_t": hs_t,
                "wa_t": np.ascontiguousarray(wa_sh.T).astype(bf),
                "wp_t": np.ascontiguousarray(wp_sh.T).astype(bf),
                "cos_t": cos_t,
                "sin_t": sin_t,
                "rot_t": rot_t.astype(bf),
                "stair_in": stair.astype(bf),
                "ones_in": ones.astype(bf),
                "ident_in": ident.astype(bf),
            }
        )
    return in_maps


def kernel(hidden_states, w_attn, w_proj, rope_cos, rope_sin, **_unused):
    nc = _get_module()
    in_maps = make_in_maps(hidden_states, w_attn, w_proj, rope_cos, rope_sin)
    res = run_bass_kernel_spmd(nc, in_maps, core_ids=list(range(NCORES)))

    out = np.empty((S, H), dtype=np.float32)
    for i in range(NCORES):
        out[:, i * P_SHARD : (i + 1) * P_SHARD] = res.results[i]["y"]
    return out.reshape(B, S, H)


# revision 20
# speedup vs baseline: 1.1892x; 1.0609x over previous
"""Trainium2 Bass kernel for fused causal GQA attention block.

Reference computation (B=1, S=2048, H=4096, NH=32, NKV=8, HD=128):
    qkv = hs @ w_attn.T; rope(q), rope(k); causal GQA attention;
    out @ w_proj.T

Sharding (8 cores, tensor parallel): core i owns kv-group i = rows
[i*768, (i+1)*768) of w_attn (4 q heads + 1 k + 1 v head) and rows
[i*512, (i+1)*512) of w_proj.  Each core computes its 4 heads of
attention output transposed (feature-major); a seq-chunked AllGather
assembles attnT on every core, and each core computes its 512 output
columns of the final projection per seq chunk.

v2 design (vs the f32r baseline):
  * all matmul/DMA data in bf16 (host-converted) -- halves HBM and
    collective traffic and the LDWEIGHTS stream; psums stay fp32.
  * RoPE + v-transposes interleaved into phase A per 512-seq block so
    attention starts immediately after the last qkv block.
  * attention runs seq chunks in order iq=3,2,1,0 (largest first) with
    each chunk's AllGather issued right away and its c_proj placed one
    chunk later, so every collective hides under compute.
  * causal mask applied on the PE as a staircase bias-matmul into the
    scores psum (lhsT=identity) -- no DVE hop between exp and l/o.
  * scores software-pipelined 3 deep over key tiles; exp on ACT.
  * softmax reciprocal via reciprocal_approx_fast (~5x faster).
"""

import sys

sys.path.insert(0, "/opt/trn_rl_repo")

import ml_dtypes
import numpy as np

import concourse.bass as bass
import concourse.tile as tile
from concourse import bacc, mybir
from concourse.bass_utils import run_bass_kernel_spmd

F32 = mybir.dt.float32
BF16 = mybir.dt.bfloat16

B, S, H = 1, 2048, 4096
NH, NKV, HD = 32, 8, 128
GROUP = NH // NKV  # 4
SCALE = 0.08838834764831845
NCORES = 8

M_SHARD = (GROUP + 2) * HD  # 768 rows of w_attn per core
P_SHARD = H // NCORES  # 512 rows of w_proj per core

KC = H // 128  # 32 contraction chunks of the model dim
NB = S // 512  # 4 seq blocks of 512
MT = M_SHARD // 128  # 6 row tiles of qkv_t
ST = S // 128  # 16 seq tiles of 128
NEG = -1.0e9


def build_module() -> bass.Bass:
    nc = bacc.Bacc(
        "TRN2",
        target_bir_lowering=False,
        debug=False,
        num_devices=NCORES,
    )

    hs_t = nc.dram_tensor("hs_t", [H, S], BF16, kind="ExternalInput")
    wa_t = nc.dram_tensor("wa_t", [H, M_SHARD], BF16, kind="ExternalInput")
    wp_t = nc.dram_tensor("wp_t", [H, P_SHARD], BF16, kind="ExternalInput")
    cos_t = nc.dram_tensor("cos_t", [HD, S], BF16, kind="ExternalInput")
    sin_t = nc.dram_tensor("sin_t", [HD, S], BF16, kind="ExternalInput")
    rot_t = nc.dram_tensor("rot_t", [HD, HD], BF16, kind="ExternalInput")
    stair_in = nc.dram_tensor("stair_in", [128, 512], BF16, kind="ExternalInput")
    ones_in = nc.dram_tensor("ones_in", [128, 128], BF16, kind="ExternalInput")
    ident_in = nc.dram_tensor("ident_in", [128, 128], BF16, kind="ExternalInput")
    y_out = nc.dram_tensor("y", [S, P_SHARD], F32, kind="ExternalOutput")

    # per-seq-chunk collective buffers
    ag_ins = [
        nc.dram_tensor(f"ag_in{i}", [GROUP * HD, 512], BF16, kind="Internal")
        for i in range(NB)
    ]
    ag_outs = [
        nc.dram_tensor(
            f"ag_out{i}", [H, 512], BF16, kind="Internal", addr_space="Shared"
        )
        for i in range(NB)
    ]

    # DRAM views with 128-partition tiling of the contraction axis
    hs_v = hs_t[:].rearrange("(ko p) n -> p ko n", p=128)  # [128, 32, 2048]
    wa_v = wa_t[:].rearrange("(ko p) m -> p ko m", p=128)  # [128, 32, 768]
    wp_v = wp_t[:].rearrange("(ko p) m -> p ko m", p=128)  # [128, 32, 512]
    ag_vs = [a[:].rearrange("(ko p) n -> p ko n", p=128) for a in ag_outs]

    with tile.TileContext(nc) as tc:
        # ---------- persistent pools ----------
        const_pool = tc.alloc_tile_pool(name="consts", bufs=1)
        qkv_pool = tc.alloc_tile_pool(name="qkv", bufs=1)
        vnat_pool = tc.alloc_tile_pool(name="vnat", bufs=1)
        wp_pool = tc.alloc_tile_pool(name="wp", bufs=1)

        ones_sb = const_pool.tile([128, 128], BF16)
        ident_sb = const_pool.tile([128, 128], BF16)
        rot_sb = const_pool.tile([128, HD], BF16)
        stair_sb = const_pool.tile([128, 512], BF16)

        qkv_sb = qkv_pool.tile([128, MT, S], BF16)  # 24KB/part
        v_nat = vnat_pool.tile([128, ST, HD], BF16)  # 4KB/part
        wp_sb = wp_pool.tile([128, KC, P_SHARD], BF16)  # 32KB/part

        # ---------- phase A: qkv_t = wa_shard @ hs.T, rope fused ----------
        with (
            tc.tile_pool(name="wa", bufs=1) as wa_pool,
            tc.tile_pool(name="hs", bufs=3) as hs_pool,
            tc.tile_pool(name="rope", bufs=1) as rope_pool,
            tc.tile_pool(name="rs", bufs=2) as rs_pool,
            tc.tile_pool(name="psA", bufs=1, space="PSUM") as psA,
            tc.tile_pool(name="psR", bufs=2, space="PSUM") as psR,
        ):
            wa_sb = wa_pool.tile([128, KC, M_SHARD], BF16)  # 48KB/part
            cos_sb = rope_pool.tile([128, S], BF16, tag="cos")
            sin_sb = rope_pool.tile([128, S], BF16, tag="sin")

            # wa chunks first on scalar (PE needs chunk0 immediately); the
            # first hs slab is first on sync.  Constants / rope tables /
            # w_proj stream behind — they're needed much later.
            for k0, k1 in ((0, 4), (4, 8), (8, 16), (16, 24), (24, 32)):
                nc.scalar.dma_start(
                    out=wa_sb[:, k0:k1, :], in_=wa_v[:, k0:k1, :]
                )
            nc.scalar.dma_start(out=rot_sb, in_=rot_t[:])
            nc.scalar.dma_start(out=ones_sb, in_=ones_in[:])
            nc.scalar.dma_start(out=ident_sb, in_=ident_in[:])
            nc.scalar.dma_start(out=stair_sb, in_=stair_in[:])
            for kk in range(0, KC, 8):
                nc.scalar.dma_start(
                    out=wp_sb[:, kk : kk + 8, :], in_=wp_v[:, kk : kk + 8, :]
                )

            # slab list; nb0 starts with two small slabs so the PE can begin
            # within ~2us of kernel start
            slabs = [(0, 0, 4), (0, 4, 8), (0, 8, 16), (0, 16, 24), (0, 24, 32)]
            for nb in range(1, NB):
                slabs += [(nb, ks, ks + 8) for ks in range(0, KC, 8)]
            slab_tiles: dict[int, bass.AP] = {}

            def issue_slab(i):
                nb, k0, k1 = slabs[i]
                t = hs_pool.tile([128, 8, 512], BF16, name="hs_slab")
                nc.sync.dma_start(
                    out=t[:, 0 : k1 - k0, :],
                    in_=hs_v[:, k0:k1, nb * 512 : (nb + 1) * 512],
                )
                slab_tiles[i] = t

            issue_slab(0)
            issue_slab(1)
            nc.sync.dma_start(out=cos_sb, in_=cos_t[:])
            nc.sync.dma_start(out=sin_sb, in_=sin_t[:])

            psums = None
            for i, (nb, k0, k1) in enumerate(slabs):
                blk = slice(nb * 512, (nb + 1) * 512)
                if k0 == 0:
                    psums = [
                        psA.tile([128, 512], F32, tag=f"ps{m}", name=f"psA{m}")
                        for m in range(MT)
                    ]
                hs_slab = slab_tiles.pop(i)
                for k in range(k0, k1):
                    for m in range(MT):
                        nc.tensor.matmul(
                            psums[m],
                            lhsT=wa_sb[:, k, m * 128 : (m + 1) * 128],
                            rhs=hs_slab[:, k - k0, :],
                            start=(k == 0),
                            stop=(k == KC - 1),
                        )
                # keep two slabs in flight ahead of the consumer so the
                # per-nb copies/transposes/rope never delay the stream
                if i + 2 < len(slabs):
                    issue_slab(i + 2)
                if k1 != KC:
                    continue
                for m in range(MT):
                    nc.vector.tensor_copy(out=qkv_sb[:, m, blk], in_=psums[m])

                # v natural layout via DMA-transpose (sync queue, tiny)
                for u in range(4):
                    nc.sync.dma_start_transpose(
                        v_nat[:, nb * 4 + u, :],
                        qkv_sb[:, GROUP + 1, nb * 512 + u * 128 : nb * 512 + (u + 1) * 128],
                    )

                # rope this seq block, k tile first (attention needs kT whole)
                for t in (GROUP, 0, 1, 2, 3):
                    x = qkv_sb[:, t, blk]
                    rp = psR.tile([128, 512], F32, name="rp")
                    nc.tensor.matmul(rp, lhsT=rot_sb[:], rhs=x, start=True, stop=True)
                    rs = rs_pool.tile([128, 512], BF16, name="rs")
                    nc.vector.tensor_mul(rs, rp, sin_sb[:, blk])
                    nc.vector.tensor_mul(x, x, cos_sb[:, blk])
                    nc.vector.tensor_add(x, x, rs)

        # ---------- phase B: attention + chunked AG + c_proj ----------
        with (
            tc.tile_pool(name="pt", bufs=1) as pt_pool,
            tc.tile_pool(name="attn", bufs=2) as attn_pool,
            tc.tile_pool(name="lh", bufs=4) as lh_pool,
            tc.tile_pool(name="ysb", bufs=2) as y_pool,
            tc.tile_pool(name="psS", bufs=3, space="PSUM") as psS,
            tc.tile_pool(name="psL", bufs=1, space="PSUM") as psL,
            tc.tile_pool(name="psO", bufs=2, space="PSUM") as psO,
            tc.tile_pool(name="psC", bufs=2, space="PSUM") as psC,
        ):
            # pre-zero the shifted-diagonal pt tags' dead zones once
            for r in (128, 256, 384):
                ptd = pt_pool.tile(
                    [128, 512], BF16, tag=f"ptd{r}", name=f"ptd{r}"
                )
                nc.vector.memset(ptd[:, 0:r], 0.0)

            kT = qkv_sb[:, GROUP, :]
            lh_tiles = {}

            def emit_lh(iq):
                # gpsimd SWDGE: the only other gpsimd work is the collective
                # triggers, whose waits resolve in the same order, so the
                # scheduler can't park an AllGather-blocked DMA in front of
                # anything latency-critical (it did exactly that on the
                # scalar queue: 29us exp stall; on sync it stalled the CC
                # stream itself: 96us).
                tiles = []
                for sub in range(4):
                    lh = lh_pool.tile([128, KC, 128], BF16, tag="lh", name="lh")
                    nc.gpsimd.dma_start(
                        out=lh, in_=ag_vs[iq][:, :, sub * 128 : (sub + 1) * 128]
                    )
                    tiles.append(lh)
                return tiles

            def cproj(iq, tiles):
                for sub in range(4):
                    lh = tiles[sub]
                    yp = psC.tile([128, 512], F32, name="yp")
                    for k in range(KC):
                        nc.tensor.matmul(
                            yp,
                            lhsT=lh[:, k, :],
                            rhs=wp_sb[:, k, :],
                            start=(k == 0),
                            stop=(k == KC - 1),
                        )
                    ysb = y_pool.tile([128, P_SHARD], F32, name="ysb")
                    nc.vector.tensor_copy(out=ysb, in_=yp)
                    nc.sync.dma_start(
                        out=y_out[(iq * 4 + sub) * 128 : (iq * 4 + sub + 1) * 128, :],
                        in_=ysb,
                    )

            order = [3, 2, 1, 0]
            for idx, iq in enumerate(order):
                njb = 4 * iq + 4
                q0 = iq * 512
                for h in range(GROUP):
                    l_ps = psL.tile([128, 512], F32, name="l_ps")
                    o_ps = psO.tile([128, 512], F32, name="o_ps")

                    # round r: key tile j=r; last 4 rounds are diagonal with
                    # shifted q-range [rel, 512) and a staircase bias matmul
                    sts = []  # per round: (st psum, pt tile, rel)

                    def emit_st(r):
                        j = r
                        rel = max(0, (j - 4 * iq) * 128)
                        st = psS.tile([128, 512], F32, name="st")
                        nc.tensor.matmul(
                            st[:, rel:512],
                            lhsT=kT[:, j * 128 : (j + 1) * 128],
                            rhs=qkv_sb[:, h, q0 + rel : q0 + 512],
                            start=True,
                            stop=(rel == 0),
                        )
                        if rel > 0:
                            nc.tensor.matmul(
                                st[:, rel:512],
                                lhsT=ident_sb[:],
                                rhs=stair_sb[:, 0 : 512 - rel],
                                start=False,
                                stop=True,
                            )
                        tag = f"ptd{rel}" if rel else "pt"
                        bufs = 1 if rel else 4
                        pt = pt_pool.tile(
                            [128, 512], BF16, tag=tag, bufs=bufs, name="pt"
                        )
                        nc.scalar.activation(
                            out=pt[:, rel:512],
                            in_=st[:, rel:512],
                            func=mybir.ActivationFunctionType.Exp,
                            scale=SCALE,
                        )
                        sts.append((st, pt, rel))

                    def emit_st_diag0(r):
                        # diagonal tile at rel==0 (j == 4*iq): triangle bias
                        j = r
                        st = psS.tile([128, 512], F32, name="st")
                        nc.tensor.matmul(
                            st,
                            lhsT=kT[:, j * 128 : (j + 1) * 128],
                            rhs=qkv_sb[:, h, q0 : q0 + 512],
                            start=True,
                            stop=False,
                        )
                        nc.tensor.matmul(
                            st,
                            lhsT=ident_sb[:],
                            rhs=stair_sb[:],
                            start=False,
                            stop=True,
                        )
                        pt = pt_pool.tile([128, 512], BF16, tag="pt", bufs=4, name="pt")
                        nc.scalar.activation(
                            out=pt,
                            in_=st,
                            func=mybir.ActivationFunctionType.Exp,
                            scale=SCALE,
                        )
                        sts.append((st, pt, 0))

                    def emit_round_st(r):
                        if r == 4 * iq:
                            emit_st_diag0(r)
                        else:
                            emit_st(r)

                    def emit_lo(r):
                        _, pt, _ = sts[r]
                        j = r
                        nc.tensor.matmul(
                            l_ps,
                            lhsT=ones_sb[:],
                            rhs=pt[:],
                            start=(r == 0),
                            stop=(r == njb - 1),
                        )
                        nc.tensor.matmul(
                            o_ps,
                            lhsT=v_nat[:, j, :],
                            rhs=pt[:],
                            start=(r == 0),
                            stop=(r == njb - 1),
                        )

                    depth = min(3, njb)
                    for r in range(depth):
                        emit_round_st(r)
                    for r in range(njb):
                        emit_lo(r)
                        if r + depth < njb:
                            emit_round_st(r + depth)

                    linv = attn_pool.tile([128, 512], F32, tag="linv", name="linv")
                    nc.vector.reciprocal_approx_fast(out=linv, in_=l_ps)
                    at = attn_pool.tile([128, 512], BF16, tag="at", name="at")
                    nc.vector.tensor_mul(at, o_ps, linv)
                    nc.sync.dma_start(
                        out=ag_ins[iq][h * 128 : (h + 1) * 128, :], in_=at
                    )

                # lh loads for the chunk consumed next, BEFORE this chunk's
                # AG trigger, so the gpsimd stream is [.., lh(prev2), AG(iq)]
                # and the lh desc-gen never delays a ready collective
                if idx >= 2:
                    lh_tiles[order[idx - 2]] = emit_lh(order[idx - 2])

                # seq-chunked AllGather; overlaps the next chunk's compute
                nc.gpsimd.collective_compute(
                    "AllGather",
                    mybir.AluOpType.bypass,
                    replica_groups=[list(range(NCORES))],
                    ins=[ag_ins[iq][:]],
                    outs=[ag_outs[iq][:]],
                )

                # two-chunk-delayed c_proj: chunk iq's AllGather gets two
                # attention chunks (~60us) to complete before its consumer
                if idx >= 2:
                    cproj(order[idx - 2], lh_tiles.pop(order[idx - 2]))
            for iq in (order[-2], order[-1]):
                cproj(iq, emit_lh(iq))

        wp_pool.release()
        vnat_pool.release()
        qkv_pool.release()
        const_pool.release()

    nc.compile()
    return nc


_CACHED = {}


def _get_module():
    if "nc" not in _CACHED:
        _CACHED["nc"] = build_module()
    return _CACHED["nc"]


def make_in_maps(hidden_states, w_attn, w_proj, rope_cos, rope_sin):
    bf = ml_dtypes.bfloat16
    hidden_states = np.asarray(hidden_states, dtype=np.float32)
    w_attn = np.asarray(w_attn, dtype=np.float32)
    w_proj = np.asarray(w_proj, dtype=np.float32)
    rope_cos = np.asarray(rope_cos, dtype=np.float32)
    rope_sin = np.asarray(rope_sin, dtype=np.float32)

    hs_t = np.ascontiguousarray(hidden_states.reshape(S, H).T).astype(bf)
    cos_t = np.ascontiguousarray(rope_cos.T).astype(bf)
    sin_t = np.ascontiguousarray(rope_sin.T).astype(bf)

    # rotate-half as a matmul: rot(x) = R @ x for x in [HD, S] layout,
    # rot_t = R.T so that lhsT.T @ x = R @ x
    rot_t = np.zeros((HD, HD), dtype=np.float32)
    half = HD // 2
    rot_t[half + np.arange(half), np.arange(half)] = -1.0
    rot_t[np.arange(half), half + np.arange(half)] = 1.0

    # causal triangle bias: stair[k, u] = NEG iff u < k (else 0); a diag
    # key tile at relative offset rel uses cols [0, 512-rel)
    kk, uu = np.meshgrid(np.arange(128), np.arange(512), indexing="ij")
    stair = np.where(uu < kk, NEG, 0.0).astype(np.float32)

    ones = np.ones((128, 128), dtype=np.float32)
    ident = np.eye(128, dtype=np.float32)

    in_maps = []
    for i in range(NCORES):
        wa_sh = w_attn[i * M_SHARD : (i + 1) * M_SHARD, :]
        wp_sh = w_proj[i * P_SHARD : (i + 1) * P_SHARD, :]
        in_maps.append(
            {
                "hs_t": hs_t,
                "wa_t": np.ascontiguousarray(wa_sh.T).astype(bf),
                "wp_t": np.ascontiguousarray(wp_sh.T).astype(bf),
                "cos_t": cos_t,
                "sin_t": sin_t,
                "rot_t": rot_t.astype(bf),
                "stair_in": stair.astype(bf),
                "ones_in": ones.astype(bf),
                "ident_in": ident.astype(bf),
            }
        )
    return in_maps


def kernel(hidden_states, w_attn, w_proj, rope_cos, rope_sin, **_unused):
    nc = _get_module()
    in_maps = make_in_maps(hidden_states, w_attn, w_proj, rope_cos, rope_sin)
    res = run_bass_kernel_spmd(nc, in_maps, core_ids=list(range(NCORES)))

    out = np.empty((S, H), dtype=np.float32)
    for i in range(NCORES):
        out[:, i * P_SHARD : (i + 1) * P_SHARD] = res.results[i]["y"]
    return out.reshape(B, S, H)


# revision 21
# speedup vs baseline: 1.1968x; 1.0064x over previous
"""Trainium2 Bass kernel for fused causal GQA attention block.

Reference computation (B=1, S=2048, H=4096, NH=32, NKV=8, HD=128):
    qkv = hs @ w_attn.T; rope(q), rope(k); causal GQA attention;
    out @ w_proj.T

Sharding (8 cores, tensor parallel): core i owns kv-group i = rows
[i*768, (i+1)*768) of w_attn (4 q heads + 1 k + 1 v head) and rows
[i*512, (i+1)*512) of w_proj.  Each core computes its 4 heads of
attention output transposed (feature-major); a seq-chunked AllGather
assembles attnT on every core, and each core computes its 512 output
columns of the final projection per seq chunk.

v2 design (vs the f32r baseline):
  * all matmul/DMA data in bf16 (host-converted) -- halves HBM and
    collective traffic and the LDWEIGHTS stream; psums stay fp32.
  * RoPE + v-transposes interleaved into phase A per 512-seq block so
    attention starts immediately after the last qkv block.
  * attention runs seq chunks in order iq=3,2,1,0 (largest first) with
    each chunk's AllGather issued right away and its c_proj placed one
    chunk later, so every collective hides under compute.
  * causal mask applied on the PE as a staircase bias-matmul into the
    scores psum (lhsT=identity) -- no DVE hop between exp and l/o.
  * scores software-pipelined 3 deep over key tiles; exp on ACT.
  * softmax reciprocal via reciprocal_approx_fast (~5x faster).
"""

import sys

sys.path.insert(0, "/opt/trn_rl_repo")

import ml_dtypes
import numpy as np

import concourse.bass as bass
import concourse.tile as tile
from concourse import bacc, mybir
from concourse.bass_utils import run_bass_kernel_spmd

F32 = mybir.dt.float32
BF16 = mybir.dt.bfloat16

B, S, H = 1, 2048, 4096
NH, NKV, HD = 32, 8, 128
GROUP = NH // NKV  # 4
SCALE = 0.08838834764831845
NCORES = 8

M_SHARD = (GROUP + 2) * HD  # 768 rows of w_attn per core
P_SHARD = H // NCORES  # 512 rows of w_proj per core

KC = H // 128  # 32 contraction chunks of the model dim
NB = S // 512  # 4 seq blocks of 512
MT = M_SHARD // 128  # 6 row tiles of qkv_t
ST = S // 128  # 16 seq tiles of 128
NEG = -1.0e9


def build_module() -> bass.Bass:
    nc = bacc.Bacc(
        "TRN2",
        target_bir_lowering=False,
        debug=False,
        num_devices=NCORES,
    )

    hs_t = nc.dram_tensor("hs_t", [H, S], BF16, kind="ExternalInput")
    wa_t = nc.dram_tensor("wa_t", [H, M_SHARD], BF16, kind="ExternalInput")
    wp_t = nc.dram_tensor("wp_t", [H, P_SHARD], BF16, kind="ExternalInput")
    cos_t = nc.dram_tensor("cos_t", [HD, S], BF16, kind="ExternalInput")
    sin_t = nc.dram_tensor("sin_t", [HD, S], BF16, kind="ExternalInput")
    rot_t = nc.dram_tensor("rot_t", [HD, HD], BF16, kind="ExternalInput")
    stair_in = nc.dram_tensor("stair_in", [128, 512], BF16, kind="ExternalInput")
    ones_in = nc.dram_tensor("ones_in", [128, 128], BF16, kind="ExternalInput")
    ident_in = nc.dram_tensor("ident_in", [128, 128], BF16, kind="ExternalInput")
    y_out = nc.dram_tensor("y", [S, P_SHARD], F32, kind="ExternalOutput")

    # per-seq-chunk collective buffers
    ag_ins = [
        nc.dram_tensor(f"ag_in{i}", [GROUP * HD, 512], BF16, kind="Internal")
        for i in range(NB)
    ]
    ag_outs = [
        nc.dram_tensor(
            f"ag_out{i}", [H, 512], BF16, kind="Internal", addr_space="Shared"
        )
        for i in range(NB)
    ]

    # DRAM views with 128-partition tiling of the contraction axis
    hs_v = hs_t[:].rearrange("(ko p) n -> p ko n", p=128)  # [128, 32, 2048]
    wa_v = wa_t[:].rearrange("(ko p) m -> p ko m", p=128)  # [128, 32, 768]
    wp_v = wp_t[:].rearrange("(ko p) m -> p ko m", p=128)  # [128, 32, 512]
    ag_vs = [a[:].rearrange("(ko p) n -> p ko n", p=128) for a in ag_outs]

    with tile.TileContext(nc) as tc:
        # ---------- persistent pools ----------
        const_pool = tc.alloc_tile_pool(name="consts", bufs=1)
        qkv_pool = tc.alloc_tile_pool(name="qkv", bufs=1)
        vnat_pool = tc.alloc_tile_pool(name="vnat", bufs=1)
        wp_pool = tc.alloc_tile_pool(name="wp", bufs=1)

        ones_sb = const_pool.tile([128, 128], BF16)
        ident_sb = const_pool.tile([128, 128], BF16)
        rot_sb = const_pool.tile([128, HD], BF16)
        stair_sb = const_pool.tile([128, 512], BF16)

        qkv_sb = qkv_pool.tile([128, MT, S], BF16)  # 24KB/part
        v_nat = vnat_pool.tile([128, ST, HD], BF16)  # 4KB/part
        wp_sb = wp_pool.tile([128, KC, P_SHARD], BF16)  # 32KB/part

        # ---------- phase A: qkv_t = wa_shard @ hs.T, rope fused ----------
        with (
            tc.tile_pool(name="wa", bufs=1) as wa_pool,
            tc.tile_pool(name="hs", bufs=3) as hs_pool,
            tc.tile_pool(name="rope", bufs=1) as rope_pool,
            tc.tile_pool(name="rs", bufs=2) as rs_pool,
            tc.tile_pool(name="psA", bufs=1, space="PSUM") as psA,
            tc.tile_pool(name="psR", bufs=2, space="PSUM") as psR,
        ):
            wa_sb = wa_pool.tile([128, KC, M_SHARD], BF16)  # 48KB/part
            cos_sb = rope_pool.tile([128, S], BF16, tag="cos")
            sin_sb = rope_pool.tile([128, S], BF16, tag="sin")

            # wa chunks first on scalar (PE needs chunk0 immediately); the
            # first hs slab is first on sync.  Constants / rope tables /
            # w_proj stream behind — they're needed much later.
            for k0, k1 in ((0, 4), (4, 8), (8, 16), (16, 24), (24, 32)):
                nc.scalar.dma_start(
                    out=wa_sb[:, k0:k1, :], in_=wa_v[:, k0:k1, :]
                )
            nc.scalar.dma_start(out=rot_sb, in_=rot_t[:])
            nc.scalar.dma_start(out=ones_sb, in_=ones_in[:])
            nc.scalar.dma_start(out=ident_sb, in_=ident_in[:])
            nc.scalar.dma_start(out=stair_sb, in_=stair_in[:])
            for kk in range(0, KC, 8):
                nc.scalar.dma_start(
                    out=wp_sb[:, kk : kk + 8, :], in_=wp_v[:, kk : kk + 8, :]
                )

            # slab list; nb0 starts with two small slabs so the PE can begin
            # within ~2us of kernel start
            slabs = [(0, 0, 4), (0, 4, 8), (0, 8, 16), (0, 16, 24), (0, 24, 32)]
            for nb in range(1, NB):
                slabs += [(nb, ks, ks + 8) for ks in range(0, KC, 8)]
            slab_tiles: dict[int, bass.AP] = {}

            def issue_slab(i):
                nb, k0, k1 = slabs[i]
                t = hs_pool.tile([128, 8, 512], BF16, name="hs_slab")
                nc.sync.dma_start(
                    out=t[:, 0 : k1 - k0, :],
                    in_=hs_v[:, k0:k1, nb * 512 : (nb + 1) * 512],
                )
                slab_tiles[i] = t

            issue_slab(0)
            issue_slab(1)
            nc.sync.dma_start(out=cos_sb, in_=cos_t[:])
            nc.sync.dma_start(out=sin_sb, in_=sin_t[:])

            psums = None
            for i, (nb, k0, k1) in enumerate(slabs):
                blk = slice(nb * 512, (nb + 1) * 512)
                if k0 == 0:
                    psums = [
                        psA.tile([128, 512], F32, tag=f"ps{m}", name=f"psA{m}")
                        for m in range(MT)
                    ]
                hs_slab = slab_tiles.pop(i)
                for k in range(k0, k1):
                    for m in range(MT):
                        nc.tensor.matmul(
                            psums[m],
                            lhsT=wa_sb[:, k, m * 128 : (m + 1) * 128],
                            rhs=hs_slab[:, k - k0, :],
                            start=(k == 0),
                            stop=(k == KC - 1),
                        )
                # keep two slabs in flight ahead of the consumer so the
                # per-nb copies/transposes/rope never delay the stream
                if i + 2 < len(slabs):
                    issue_slab(i + 2)
                if k1 != KC:
                    continue
                for m in range(MT):
                    nc.vector.tensor_copy(out=qkv_sb[:, m, blk], in_=psums[m])

                # v natural layout via DMA-transpose (sync queue, tiny)
                for u in range(4):
                    nc.sync.dma_start_transpose(
                        v_nat[:, nb * 4 + u, :],
                        qkv_sb[:, GROUP + 1, nb * 512 + u * 128 : nb * 512 + (u + 1) * 128],
                    )

                # rope this seq block, k tile first (attention needs kT whole)
                for t in (GROUP, 0, 1, 2, 3):
                    x = qkv_sb[:, t, blk]
                    rp = psR.tile([128, 512], F32, name="rp")
                    nc.tensor.matmul(rp, lhsT=rot_sb[:], rhs=x, start=True, stop=True)
                    rs = rs_pool.tile([128, 512], BF16, name="rs")
                    nc.vector.tensor_mul(rs, rp, sin_sb[:, blk])
                    nc.vector.tensor_mul(x, x, cos_sb[:, blk])
                    nc.vector.tensor_add(x, x, rs)

        # ---------- phase B: attention + chunked AG + c_proj ----------
        with (
            tc.tile_pool(name="pt", bufs=1) as pt_pool,
            tc.tile_pool(name="attn", bufs=2) as attn_pool,
            tc.tile_pool(name="lh", bufs=4) as lh_pool,
            tc.tile_pool(name="ysb", bufs=2) as y_pool,
            tc.tile_pool(name="psS", bufs=3, space="PSUM") as psS,
            tc.tile_pool(name="psL", bufs=1, space="PSUM") as psL,
            tc.tile_pool(name="psO", bufs=2, space="PSUM") as psO,
            tc.tile_pool(name="psC", bufs=2, space="PSUM") as psC,
        ):
            # pre-zero the shifted-diagonal pt tags' dead zones once
            for r in (128, 256, 384):
                ptd = pt_pool.tile(
                    [128, 512], BF16, tag=f"ptd{r}", name=f"ptd{r}"
                )
                nc.vector.memset(ptd[:, 0:r], 0.0)

            kT = qkv_sb[:, GROUP, :]
            lh_tiles = {}

            def emit_lh(iq):
                # gpsimd SWDGE: the only other gpsimd work is the collective
                # triggers, whose waits resolve in the same order, so the
                # scheduler can't park an AllGather-blocked DMA in front of
                # anything latency-critical (it did exactly that on the
                # scalar queue: 29us exp stall; on sync it stalled the CC
                # stream itself: 96us).
                tiles = []
                for sub in range(4):
                    lh = lh_pool.tile([128, KC, 128], BF16, tag="lh", name="lh")
                    nc.gpsimd.dma_start(
                        out=lh, in_=ag_vs[iq][:, :, sub * 128 : (sub + 1) * 128]
                    )
                    tiles.append(lh)
                return tiles

            def cproj(iq, tiles):
                for sub in range(4):
                    lh = tiles[sub]
                    yp = psC.tile([128, 512], F32, name="yp")
                    for k in range(KC):
                        nc.tensor.matmul(
                            yp,
                            lhsT=lh[:, k, :],
                            rhs=wp_sb[:, k, :],
                            start=(k == 0),
                            stop=(k == KC - 1),
                        )
                    ysb = y_pool.tile([128, P_SHARD], F32, name="ysb")
                    nc.vector.tensor_copy(out=ysb, in_=yp)
                    nc.sync.dma_start(
                        out=y_out[(iq * 4 + sub) * 128 : (iq * 4 + sub + 1) * 128, :],
                        in_=ysb,
                    )

            order = [3, 2, 1, 0]
            for idx, iq in enumerate(order):
                njb = 4 * iq + 4
                q0 = iq * 512
                for h in range(GROUP):
                    l_ps = psL.tile([128, 512], F32, name="l_ps")
                    o_ps = psO.tile([128, 512], F32, name="o_ps")

                    # round r: key tile j=r; last 4 rounds are diagonal with
                    # shifted q-range [rel, 512) and a staircase bias matmul
                    sts = []  # per round: (st psum, pt tile, rel)

                    def emit_st(r):
                        j = r
                        rel = max(0, (j - 4 * iq) * 128)
                        st = psS.tile([128, 512], F32, name="st")
                        nc.tensor.matmul(
                            st[:, rel:512],
                            lhsT=kT[:, j * 128 : (j + 1) * 128],
                            rhs=qkv_sb[:, h, q0 + rel : q0 + 512],
                            start=True,
                            stop=(rel == 0),
                        )
                        if rel > 0:
                            nc.tensor.matmul(
                                st[:, rel:512],
                                lhsT=ident_sb[:],
                                rhs=stair_sb[:, 0 : 512 - rel],
                                start=False,
                                stop=True,
                            )
                        tag = f"ptd{rel}" if rel else "pt"
                        bufs = 1 if rel else 4
                        pt = pt_pool.tile(
                            [128, 512], BF16, tag=tag, bufs=bufs, name="pt"
                        )
                        nc.scalar.activation(
                            out=pt[:, rel:512],
                            in_=st[:, rel:512],
                            func=mybir.ActivationFunctionType.Exp,
                            scale=SCALE,
                        )
                        sts.append((st, pt, rel))

                    def emit_st_diag0(r):
                        # diagonal tile at rel==0 (j == 4*iq): triangle bias
                        j = r
                        st = psS.tile([128, 512], F32, name="st")
                        nc.tensor.matmul(
                            st,
                            lhsT=kT[:, j * 128 : (j + 1) * 128],
                            rhs=qkv_sb[:, h, q0 : q0 + 512],
                            start=True,
                            stop=False,
                        )
                        nc.tensor.matmul(
                            st,
                            lhsT=ident_sb[:],
                            rhs=stair_sb[:],
                            start=False,
                            stop=True,
                        )
                        pt = pt_pool.tile([128, 512], BF16, tag="pt", bufs=4, name="pt")
                        nc.scalar.activation(
                            out=pt,
                            in_=st,
                            func=mybir.ActivationFunctionType.Exp,
                            scale=SCALE,
                        )
                        sts.append((st, pt, 0))

                    def emit_round_st(r):
                        if r == 4 * iq:
                            emit_st_diag0(r)
                        else:
                            emit_st(r)

                    def emit_lo(r):
                        _, pt, _ = sts[r]
                        j = r
                        nc.tensor.matmul(
                            l_ps,
                            lhsT=ones_sb[:],
                            rhs=pt[:],
                            start=(r == 0),
                            stop=(r == njb - 1),
                        )
                        nc.tensor.matmul(
                            o_ps,
                            lhsT=v_nat[:, j, :],
                            rhs=pt[:],
                            start=(r == 0),
                            stop=(r == njb - 1),
                        )

                    depth = min(3, njb)
                    for r in range(depth):
                        emit_round_st(r)
                    for r in range(njb):
                        emit_lo(r)
                        if r + depth < njb:
                            emit_round_st(r + depth)

                    linv = attn_pool.tile([128, 512], F32, tag="linv", name="linv")
                    nc.vector.reciprocal_approx_fast(out=linv, in_=l_ps)
                    at = attn_pool.tile([128, 512], BF16, tag="at", name="at")
                    nc.vector.tensor_mul(at, o_ps, linv)
                    nc.sync.dma_start(
                        out=ag_ins[iq][h * 128 : (h + 1) * 128, :], in_=at
                    )

                # seq-chunked AllGather; overlaps the remaining compute
                nc.gpsimd.collective_compute(
                    "AllGather",
                    mybir.AluOpType.bypass,
                    replica_groups=[list(range(NCORES))],
                    ins=[ag_ins[iq][:]],
                    outs=[ag_outs[iq][:]],
                )

            # c_projs after all attention, pinned late in the scheduler's
            # model time (tile_wait_until) so it cannot hoist them ahead of
            # attention work: their lh loads wait on AllGathers whose ~45us
            # trigger-to-done latency the scheduler does not model (measured
            # 26-29us PE stalls from exactly that hoist)
            t_cproj = {3: 0.38, 2: 0.42, 1: 0.46, 0: 0.50}
            for iq in order:
                with tc.tile_wait_until(t_cproj[iq]):
                    cproj(iq, emit_lh(iq))

        wp_pool.release()
        vnat_pool.release()
        qkv_pool.release()
        const_pool.release()

    nc.compile()
    return nc


_CACHED = {}


def _get_module():
    if "nc" not in _CACHED:
        _CACHED["nc"] = build_module()
    return _CACHED["nc"]


def make_in_maps(hidden_states, w_attn, w_proj, rope_cos, rope_sin):
    bf = ml_dtypes.bfloat16
    hidden_states = np.asarray(hidden_states, dtype=np.float32)
    w_attn = np.asarray(w_attn, dtype=np.float32)
    w_proj = np.asarray(w_proj, dtype=np.float32)
    rope_cos = np.asarray(rope_cos, dtype=np.float32)
    rope_sin = np.asarray(rope_sin, dtype=np.float32)

    hs_t = np.ascontiguousarray(hidden_states.reshape(S, H).T).astype(bf)
    cos_t = np.ascontiguousarray(rope_cos.T).astype(bf)
    sin_t = np.ascontiguousarray(rope_sin.T).astype(bf)

    # rotate-half as a matmul: rot(x) = R @ x for x in [HD, S] layout,
    # rot_t = R.T so that lhsT.T @ x = R @ x
    rot_t = np.zeros((HD, HD), dtype=np.float32)
    half = HD // 2
    rot_t[half + np.arange(half), np.arange(half)] = -1.0
    rot_t[np.arange(half), half + np.arange(half)] = 1.0

    # causal triangle bias: stair[k, u] = NEG iff u < k (else 0); a diag
    # key tile at relative offset rel uses cols [0, 512-rel)
    kk, uu = np.meshgrid(np.arange(128), np.arange(512), indexing="ij")
    stair = np.where(uu < kk, NEG, 0.0).astype(np.float32)

    ones = np.ones((128, 128), dtype=np.float32)
    ident = np.eye(128, dtype=np.float32)

    in_maps = []
    for i in range(NCORES):
        wa_sh = w_attn[i * M_SHARD : (i + 1) * M_SHARD, :]
        wp_sh = w_proj[i * P_SHARD : (i + 1) * P_SHARD, :]
        in_maps.append(
            {
                "hs_t": hs_t,
                "wa_t": np.ascontiguousarray(wa_sh.T).astype(bf),
                "wp_t": np.ascontiguousarray(wp_sh.T).astype(bf),
                "cos_t": cos_t,
                "sin_t": sin_t,
                "rot_t": rot_t.astype(bf),
                "stair_in": stair.astype(bf),
                "ones_in": ones.astype(bf),
                "ident_in": ident.astype(bf),
            }
        )
    return in_maps


def kernel(hidden_states, w_attn, w_proj, rope_cos, rope_sin, **_unused):
    nc = _get_module()
    in_maps = make_in_maps(hidden_states, w_attn, w_proj, rope_cos, rope_sin)
    res = run_bass_kernel_spmd(nc, in_maps, core_ids=list(range(NCORES)))

    out = np.empty((S, H), dtype=np.float32)
    for i in range(NCORES):
        out[:, i * P_SHARD : (i + 1) * P_SHARD] = res.results[i]["y"]
    return out.reshape(B, S, H)


# revision 23
# speedup vs baseline: 1.2414x; 1.0373x over previous
"""Trainium2 Bass kernel for fused causal GQA attention block.

Reference computation (B=1, S=2048, H=4096, NH=32, NKV=8, HD=128):
    qkv = hs @ w_attn.T; rope(q), rope(k); causal GQA attention;
    out @ w_proj.T

Sharding (8 cores, tensor parallel): core i owns kv-group i = rows
[i*768, (i+1)*768) of w_attn (4 q heads + 1 k + 1 v head) and rows
[i*512, (i+1)*512) of w_proj.  Each core computes its 4 heads of
attention output transposed (feature-major); a seq-chunked AllGather
assembles attnT on every core, and each core computes its 512 output
columns of the final projection per seq chunk.

v2 design (vs the f32r baseline):
  * all matmul/DMA data in bf16 (host-converted) -- halves HBM and
    collective traffic and the LDWEIGHTS stream; psums stay fp32.
  * RoPE + v-transposes interleaved into phase A per 512-seq block so
    attention starts immediately after the last qkv block.
  * attention runs seq chunks in order iq=3,2,1,0 (largest first) with
    each chunk's AllGather issued right away and its c_proj placed one
    chunk later, so every collective hides under compute.
  * causal mask applied on the PE as a staircase bias-matmul into the
    scores psum (lhsT=identity) -- no DVE hop between exp and l/o.
  * scores software-pipelined 3 deep over key tiles; exp on ACT.
  * softmax reciprocal via reciprocal_approx_fast (~5x faster).
"""

import sys

sys.path.insert(0, "/opt/trn_rl_repo")

import ml_dtypes
import numpy as np

import concourse.bass as bass
import concourse.tile as tile
from concourse import bacc, mybir
from concourse.bass_utils import run_bass_kernel_spmd

F32 = mybir.dt.float32
BF16 = mybir.dt.bfloat16

B, S, H = 1, 2048, 4096
NH, NKV, HD = 32, 8, 128
GROUP = NH // NKV  # 4
SCALE = 0.08838834764831845
NCORES = 8

M_SHARD = (GROUP + 2) * HD  # 768 rows of w_attn per core
P_SHARD = H // NCORES  # 512 rows of w_proj per core

KC = H // 128  # 32 contraction chunks of the model dim
NB = S // 512  # 4 seq blocks of 512
MT = M_SHARD // 128  # 6 row tiles of qkv_t
ST = S // 128  # 16 seq tiles of 128
NEG = -1.0e9


def build_module() -> bass.Bass:
    nc = bacc.Bacc(
        "TRN2",
        target_bir_lowering=False,
        debug=False,
        num_devices=NCORES,
    )

    hs_t = nc.dram_tensor("hs_t", [H, S], BF16, kind="ExternalInput")
    wa_t = nc.dram_tensor("wa_t", [H, M_SHARD], BF16, kind="ExternalInput")
    wp_t = nc.dram_tensor("wp_t", [H, P_SHARD], BF16, kind="ExternalInput")
    cos_t = nc.dram_tensor("cos_t", [HD, S], BF16, kind="ExternalInput")
    sin_t = nc.dram_tensor("sin_t", [HD, S], BF16, kind="ExternalInput")
    rot_t = nc.dram_tensor("rot_t", [HD, HD], BF16, kind="ExternalInput")
    stair_in = nc.dram_tensor("stair_in", [128, 512], BF16, kind="ExternalInput")
    ones_in = nc.dram_tensor("ones_in", [128, 128], BF16, kind="ExternalInput")
    ident_in = nc.dram_tensor("ident_in", [128, 128], BF16, kind="ExternalInput")
    y_out = nc.dram_tensor("y", [S, P_SHARD], BF16, kind="ExternalOutput")

    # per-seq-chunk collective buffers
    ag_ins = [
        nc.dram_tensor(f"ag_in{i}", [GROUP * HD, 512], BF16, kind="Internal")
        for i in range(NB)
    ]
    ag_outs = [
        nc.dram_tensor(
            f"ag_out{i}", [H, 512], BF16, kind="Internal", addr_space="Shared"
        )
        for i in range(NB)
    ]

    # DRAM views with 128-partition tiling of the contraction axis
    hs_v = hs_t[:].rearrange("(ko p) n -> p ko n", p=128)  # [128, 32, 2048]
    wa_v = wa_t[:].rearrange("(ko p) m -> p ko m", p=128)  # [128, 32, 768]
    wp_v = wp_t[:].rearrange("(ko p) m -> p ko m", p=128)  # [128, 32, 512]
    ag_vs = [a[:].rearrange("(ko p) n -> p ko n", p=128) for a in ag_outs]

    with tile.TileContext(nc) as tc:
        # ---------- persistent pools ----------
        const_pool = tc.alloc_tile_pool(name="consts", bufs=1)
        qkv_pool = tc.alloc_tile_pool(name="qkv", bufs=1)
        vnat_pool = tc.alloc_tile_pool(name="vnat", bufs=1)
        wp_pool = tc.alloc_tile_pool(name="wp", bufs=1)

        ones_sb = const_pool.tile([128, 128], BF16)
        ident_sb = const_pool.tile([128, 128], BF16)
        rot_sb = const_pool.tile([128, HD], BF16)
        stair_sb = const_pool.tile([128, 512], BF16)

        qkv_sb = qkv_pool.tile([128, MT, S], BF16)  # 24KB/part
        v_nat = vnat_pool.tile([128, ST, HD], BF16)  # 4KB/part
        wp_sb = wp_pool.tile([128, KC, P_SHARD], BF16)  # 32KB/part

        # ---------- phase A: qkv_t = wa_shard @ hs.T, rope fused ----------
        with (
            tc.tile_pool(name="wa", bufs=1) as wa_pool,
            tc.tile_pool(name="hs", bufs=3) as hs_pool,
            tc.tile_pool(name="rope", bufs=1) as rope_pool,
            tc.tile_pool(name="rs", bufs=2) as rs_pool,
            tc.tile_pool(name="psA", bufs=1, space="PSUM") as psA,
            tc.tile_pool(name="psR", bufs=2, space="PSUM") as psR,
        ):
            wa_sb = wa_pool.tile([128, KC, M_SHARD], BF16)  # 48KB/part
            cos_sb = rope_pool.tile([128, S], BF16, tag="cos")
            sin_sb = rope_pool.tile([128, S], BF16, tag="sin")

            # wa chunks first on scalar (PE needs chunk0 immediately); the
            # first hs slab is first on sync.  Constants / rope tables /
            # w_proj stream behind — they're needed much later.
            for k0, k1 in ((0, 4), (4, 8), (8, 16), (16, 24), (24, 32)):
                nc.scalar.dma_start(
                    out=wa_sb[:, k0:k1, :], in_=wa_v[:, k0:k1, :]
                )
            nc.scalar.dma_start(out=rot_sb, in_=rot_t[:])
            nc.scalar.dma_start(out=ones_sb, in_=ones_in[:])
            nc.scalar.dma_start(out=ident_sb, in_=ident_in[:])
            nc.scalar.dma_start(out=stair_sb, in_=stair_in[:])
            for kk in range(0, KC, 8):
                nc.scalar.dma_start(
                    out=wp_sb[:, kk : kk + 8, :], in_=wp_v[:, kk : kk + 8, :]
                )

            # slab list; nb0 starts with two small slabs so the PE can begin
            # within ~2us of kernel start
            slabs = [(0, 0, 4), (0, 4, 8), (0, 8, 16), (0, 16, 24), (0, 24, 32)]
            for nb in range(1, NB):
                slabs += [(nb, ks, ks + 8) for ks in range(0, KC, 8)]
            slab_tiles: dict[int, bass.AP] = {}

            def issue_slab(i):
                nb, k0, k1 = slabs[i]
                t = hs_pool.tile([128, 8, 512], BF16, name="hs_slab")
                nc.sync.dma_start(
                    out=t[:, 0 : k1 - k0, :],
                    in_=hs_v[:, k0:k1, nb * 512 : (nb + 1) * 512],
                )
                slab_tiles[i] = t

            issue_slab(0)
            issue_slab(1)
            nc.sync.dma_start(out=cos_sb, in_=cos_t[:])
            nc.sync.dma_start(out=sin_sb, in_=sin_t[:])

            psums = None
            for i, (nb, k0, k1) in enumerate(slabs):
                blk = slice(nb * 512, (nb + 1) * 512)
                if k0 == 0:
                    psums = [
                        psA.tile([128, 512], F32, tag=f"ps{m}", name=f"psA{m}")
                        for m in range(MT)
                    ]
                hs_slab = slab_tiles.pop(i)
                for k in range(k0, k1):
                    for m in range(MT):
                        nc.tensor.matmul(
                            psums[m],
                            lhsT=wa_sb[:, k, m * 128 : (m + 1) * 128],
                            rhs=hs_slab[:, k - k0, :],
                            start=(k == 0),
                            stop=(k == KC - 1),
                        )
                # keep two slabs in flight ahead of the consumer so the
                # per-nb copies/transposes/rope never delay the stream
                if i + 2 < len(slabs):
                    issue_slab(i + 2)
                if k1 != KC:
                    continue
                for m in range(MT):
                    nc.vector.tensor_copy(out=qkv_sb[:, m, blk], in_=psums[m])

                # v natural layout via DMA-transpose (sync queue, tiny)
                for u in range(4):
                    nc.sync.dma_start_transpose(
                        v_nat[:, nb * 4 + u, :],
                        qkv_sb[:, GROUP + 1, nb * 512 + u * 128 : nb * 512 + (u + 1) * 128],
                    )

                # rope this seq block, k tile first (attention needs kT whole)
                for t in (GROUP, 0, 1, 2, 3):
                    x = qkv_sb[:, t, blk]
                    rp = psR.tile([128, 512], F32, name="rp")
                    nc.tensor.matmul(rp, lhsT=rot_sb[:], rhs=x, start=True, stop=True)
                    rs = rs_pool.tile([128, 512], BF16, name="rs")
                    nc.vector.tensor_mul(rs, rp, sin_sb[:, blk])
                    nc.vector.tensor_mul(x, x, cos_sb[:, blk])
                    nc.vector.tensor_add(x, x, rs)

        # ---------- phase B: attention + chunked AG + c_proj ----------
        with (
            tc.tile_pool(name="pt", bufs=1) as pt_pool,
            tc.tile_pool(name="attn", bufs=2) as attn_pool,
            tc.tile_pool(name="lh", bufs=4) as lh_pool,
            tc.tile_pool(name="ysb", bufs=2) as y_pool,
            tc.tile_pool(name="psS", bufs=3, space="PSUM") as psS,
            tc.tile_pool(name="psL", bufs=1, space="PSUM") as psL,
            tc.tile_pool(name="psO", bufs=2, space="PSUM") as psO,
            tc.tile_pool(name="psC", bufs=2, space="PSUM") as psC,
        ):
            # pre-zero the shifted-diagonal pt tags' dead zones once
            for r in (128, 256, 384):
                ptd = pt_pool.tile(
                    [128, 512], BF16, tag=f"ptd{r}", name=f"ptd{r}"
                )
                nc.vector.memset(ptd[:, 0:r], 0.0)

            kT = qkv_sb[:, GROUP, :]
            lh_tiles = {}

            def emit_lh(iq):
                # gpsimd SWDGE: the only other gpsimd work is the collective
                # triggers, whose waits resolve in the same order, so the
                # scheduler can't park an AllGather-blocked DMA in front of
                # anything latency-critical (it did exactly that on the
                # scalar queue: 29us exp stall; on sync it stalled the CC
                # stream itself: 96us).
                tiles = []
                for sub in range(4):
                    lh = lh_pool.tile([128, KC, 128], BF16, tag="lh", name="lh")
                    nc.gpsimd.dma_start(
                        out=lh, in_=ag_vs[iq][:, :, sub * 128 : (sub + 1) * 128]
                    )
                    tiles.append(lh)
                return tiles

            def cproj(iq, tiles):
                for sub in range(4):
                    lh = tiles[sub]
                    yp = psC.tile([128, 512], F32, name="yp")
                    for k in range(KC):
                        nc.tensor.matmul(
                            yp,
                            lhsT=lh[:, k, :],
                            rhs=wp_sb[:, k, :],
                            start=(k == 0),
                            stop=(k == KC - 1),
                        )
                    ysb = y_pool.tile([128, P_SHARD], BF16, name="ysb")
                    nc.vector.tensor_copy(out=ysb, in_=yp)
                    nc.sync.dma_start(
                        out=y_out[(iq * 4 + sub) * 128 : (iq * 4 + sub + 1) * 128, :],
                        in_=ysb,
                    )

            order = [3, 2, 1, 0]
            for idx, iq in enumerate(order):
                njb = 4 * iq + 4
                q0 = iq * 512
                for h in range(GROUP):
                    l_ps = psL.tile([128, 512], F32, name="l_ps")
                    o_ps = psO.tile([128, 512], F32, name="o_ps")

                    # round r: key tile j=r; last 4 rounds are diagonal with
                    # shifted q-range [rel, 512) and a staircase bias matmul
                    sts = []  # per round: (st psum, pt tile, rel)

                    def emit_st(r):
                        j = r
                        rel = max(0, (j - 4 * iq) * 128)
                        st = psS.tile([128, 512], F32, name="st")
                        nc.tensor.matmul(
                            st[:, rel:512],
                            lhsT=kT[:, j * 128 : (j + 1) * 128],
                            rhs=qkv_sb[:, h, q0 + rel : q0 + 512],
                            start=True,
                            stop=(rel == 0),
                        )
                        if rel > 0:
                            nc.tensor.matmul(
                                st[:, rel:512],
                                lhsT=ident_sb[:],
                                rhs=stair_sb[:, 0 : 512 - rel],
                                start=False,
                                stop=True,
                            )
                        tag = f"ptd{rel}" if rel else "pt"
                        bufs = 1 if rel else 4
                        pt = pt_pool.tile(
                            [128, 512], BF16, tag=tag, bufs=bufs, name="pt"
                        )
                        nc.scalar.activation(
                            out=pt[:, rel:512],
                            in_=st[:, rel:512],
                            func=mybir.ActivationFunctionType.Exp,
                            scale=SCALE,
                        )
                        sts.append((st, pt, rel))

                    def emit_st_diag0(r):
                        # diagonal tile at rel==0 (j == 4*iq): triangle bias
                        j = r
                        st = psS.tile([128, 512], F32, name="st")
                        nc.tensor.matmul(
                            st,
                            lhsT=kT[:, j * 128 : (j + 1) * 128],
                            rhs=qkv_sb[:, h, q0 : q0 + 512],
                            start=True,
                            stop=False,
                        )
                        nc.tensor.matmul(
                            st,
                            lhsT=ident_sb[:],
                            rhs=stair_sb[:],
                            start=False,
                            stop=True,
                        )
                        pt = pt_pool.tile([128, 512], BF16, tag="pt", bufs=4, name="pt")
                        nc.scalar.activation(
                            out=pt,
                            in_=st,
                            func=mybir.ActivationFunctionType.Exp,
                            scale=SCALE,
                        )
                        sts.append((st, pt, 0))

                    def emit_round_st(r):
                        if r == 4 * iq:
                            emit_st_diag0(r)
                        else:
                            emit_st(r)

                    def emit_lo(r):
                        _, pt, _ = sts[r]
                        j = r
                        nc.tensor.matmul(
                            l_ps,
                            lhsT=ones_sb[:],
                            rhs=pt[:],
                            start=(r == 0),
                            stop=(r == njb - 1),
                        )
                        nc.tensor.matmul(
                            o_ps,
                            lhsT=v_nat[:, j, :],
                            rhs=pt[:],
                            start=(r == 0),
                            stop=(r == njb - 1),
                        )

                    depth = min(3, njb)
                    for r in range(depth):
                        emit_round_st(r)
                    for r in range(njb):
                        emit_lo(r)
                        if r + depth < njb:
                            emit_round_st(r + depth)

                    linv = attn_pool.tile([128, 512], F32, tag="linv", name="linv")
                    nc.vector.reciprocal_approx_fast(out=linv, in_=l_ps)
                    at = attn_pool.tile([128, 512], BF16, tag="at", name="at")
                    nc.vector.tensor_mul(at, o_ps, linv)
                    nc.sync.dma_start(
                        out=ag_ins[iq][h * 128 : (h + 1) * 128, :], in_=at
                    )

                # lh loads for the chunk whose AG completed two chunks ago,
                # emitted BEFORE this chunk's AG trigger: the gpsimd engine
                # executes in order, so lh desc-gen must not sit behind an
                # AG trigger that waits on not-yet-computed at stores
                if idx >= 2:
                    lh_tiles[order[idx - 2]] = emit_lh(order[idx - 2])

                # seq-chunked AllGather; overlaps the remaining compute
                nc.gpsimd.collective_compute(
                    "AllGather",
                    mybir.AluOpType.bypass,
                    replica_groups=[list(range(NCORES))],
                    ins=[ag_ins[iq][:]],
                    outs=[ag_outs[iq][:]],
                )
            for iq in (order[-2], order[-1]):
                lh_tiles[iq] = emit_lh(iq)

            # c_projs after all attention, pinned late in the scheduler's
            # model time (tile_wait_until) so it cannot hoist them ahead of
            # attention work: their lh loads wait on AllGathers whose ~45us
            # trigger-to-done latency the scheduler does not model (measured
            # 26-29us PE stalls from exactly that hoist)
            t_cproj = {3: 0.38, 2: 0.42, 1: 0.46, 0: 0.50}
            for iq in order:
                with tc.tile_wait_until(t_cproj[iq]):
                    cproj(iq, lh_tiles.pop(iq))

        wp_pool.release()
        vnat_pool.release()
        qkv_pool.release()
        const_pool.release()

    nc.compile()
    return nc


_CACHED = {}


def _get_module():
    if "nc" not in _CACHED:
        _CACHED["nc"] = build_module()
    return _CACHED["nc"]


def make_in_maps(hidden_states, w_attn, w_proj, rope_cos, rope_sin):
    bf = ml_dtypes.bfloat16
    hidden_states = np.asarray(hidden_states, dtype=np.float32)
    w_attn = np.asarray(w_attn, dtype=np.float32)
    w_proj = np.asarray(w_proj, dtype=np.float32)
    rope_cos = np.asarray(rope_cos, dtype=np.float32)
    rope_sin = np.asarray(rope_sin, dtype=np.float32)

    hs_t = np.ascontiguousarray(hidden_states.reshape(S, H).T).astype(bf)
    cos_t = np.ascontiguousarray(rope_cos.T).astype(bf)
    sin_t = np.ascontiguousarray(rope_sin.T).astype(bf)

    # rotate-half as a matmul: rot(x) = R @ x for x in [HD, S] layout,
    # rot_t = R.T so that lhsT.T @ x = R @ x
    rot_t = np.zeros((HD, HD), dtype=np.float32)
    half = HD // 2
    rot_t[half + np.arange(half), np.arange(half)] = -1.0
    rot_t[np.arange(half), half + np.arange(half)] = 1.0

    # causal triangle bias: stair[k, u] = NEG iff u < k (else 0); a diag
    # key tile at relative offset rel uses cols [0, 512-rel)
    kk, uu = np.meshgrid(np.arange(128), np.arange(512), indexing="ij")
    stair = np.where(uu < kk, NEG, 0.0).astype(np.float32)

    ones = np.ones((128, 128), dtype=np.float32)
    ident = np.eye(128, dtype=np.float32)

    in_maps = []
    for i in range(NCORES):
        wa_sh = w_attn[i * M_SHARD : (i + 1) * M_SHARD, :]
        wp_sh = w_proj[i * P_SHARD : (i + 1) * P_SHARD, :]
        in_maps.append(
            {
                "hs_t": hs_t,
                "wa_t": np.ascontiguousarray(wa_sh.T).astype(bf),
                "wp_t": np.ascontiguousarray(wp_sh.T).astype(bf),
                "cos_t": cos_t,
                "sin_t": sin_t,
                "rot_t": rot_t.astype(bf),
                "stair_in": stair.astype(bf),
                "ones_in": ones.astype(bf),
                "ident_in": ident.astype(bf),
            }
        )
    return in_maps


def kernel(hidden_states, w_attn, w_proj, rope_cos, rope_sin, **_unused):
    nc = _get_module()
    in_maps = make_in_maps(hidden_states, w_attn, w_proj, rope_cos, rope_sin)
    res = run_bass_kernel_spmd(nc, in_maps, core_ids=list(range(NCORES)))

    out = np.empty((S, H), dtype=np.float32)
    for i in range(NCORES):
        out[:, i * P_SHARD : (i + 1) * P_SHARD] = res.results[i]["y"].astype(
            np.float32
        )
    return out.reshape(B, S, H)


# revision 24
# speedup vs baseline: 1.2570x; 1.0125x over previous
"""Trainium2 Bass kernel for fused causal GQA attention block.

Reference computation (B=1, S=2048, H=4096, NH=32, NKV=8, HD=128):
    qkv = hs @ w_attn.T; rope(q), rope(k); causal GQA attention;
    out @ w_proj.T

Sharding (8 cores, tensor parallel): core i owns kv-group i = rows
[i*768, (i+1)*768) of w_attn (4 q heads + 1 k + 1 v head) and rows
[i*512, (i+1)*512) of w_proj.  Each core computes its 4 heads of
attention output transposed (feature-major); a seq-chunked AllGather
assembles attnT on every core, and each core computes its 512 output
columns of the final projection per seq chunk.

v2 design (vs the f32r baseline):
  * all matmul/DMA data in bf16 (host-converted) -- halves HBM and
    collective traffic and the LDWEIGHTS stream; psums stay fp32.
  * RoPE + v-transposes interleaved into phase A per 512-seq block so
    attention starts immediately after the last qkv block.
  * attention runs seq chunks in order iq=3,2,1,0 (largest first) with
    each chunk's AllGather issued right away and its c_proj placed one
    chunk later, so every collective hides under compute.
  * causal mask applied on the PE as a staircase bias-matmul into the
    scores psum (lhsT=identity) -- no DVE hop between exp and l/o.
  * scores software-pipelined 3 deep over key tiles; exp on ACT.
  * softmax reciprocal via reciprocal_approx_fast (~5x faster).
"""

import sys

sys.path.insert(0, "/opt/trn_rl_repo")

import ml_dtypes
import numpy as np

import concourse.bass as bass
import concourse.tile as tile
from concourse import bacc, mybir
from concourse.bass_utils import run_bass_kernel_spmd

F32 = mybir.dt.float32
BF16 = mybir.dt.bfloat16

B, S, H = 1, 2048, 4096
NH, NKV, HD = 32, 8, 128
GROUP = NH // NKV  # 4
SCALE = 0.08838834764831845
NCORES = 8

M_SHARD = (GROUP + 2) * HD  # 768 rows of w_attn per core
P_SHARD = H // NCORES  # 512 rows of w_proj per core

KC = H // 128  # 32 contraction chunks of the model dim
NB = S // 512  # 4 seq blocks of 512
MT = M_SHARD // 128  # 6 row tiles of qkv_t
ST = S // 128  # 16 seq tiles of 128
NEG = -1.0e9


def build_module() -> bass.Bass:
    nc = bacc.Bacc(
        "TRN2",
        target_bir_lowering=False,
        debug=False,
        num_devices=NCORES,
    )

    hs_t = nc.dram_tensor("hs_t", [H, S], BF16, kind="ExternalInput")
    wa_t = nc.dram_tensor("wa_t", [H, M_SHARD], BF16, kind="ExternalInput")
    wp_t = nc.dram_tensor("wp_t", [H, P_SHARD], BF16, kind="ExternalInput")
    cos_t = nc.dram_tensor("cos_t", [HD, S], BF16, kind="ExternalInput")
    sin_t = nc.dram_tensor("sin_t", [HD, S], BF16, kind="ExternalInput")
    rot_t = nc.dram_tensor("rot_t", [HD, HD], BF16, kind="ExternalInput")
    stair_in = nc.dram_tensor("stair_in", [128, 512], BF16, kind="ExternalInput")
    ones_in = nc.dram_tensor("ones_in", [128, 128], BF16, kind="ExternalInput")
    ident_in = nc.dram_tensor("ident_in", [128, 128], BF16, kind="ExternalInput")
    y_out = nc.dram_tensor("y", [S, P_SHARD], BF16, kind="ExternalOutput")

    # per-seq-chunk collective buffers
    ag_ins = [
        nc.dram_tensor(f"ag_in{i}", [GROUP * HD, 512], BF16, kind="Internal")
        for i in range(NB)
    ]
    ag_outs = [
        nc.dram_tensor(
            f"ag_out{i}", [H, 512], BF16, kind="Internal", addr_space="Shared"
        )
        for i in range(NB)
    ]

    # DRAM views with 128-partition tiling of the contraction axis
    hs_v = hs_t[:].rearrange("(ko p) n -> p ko n", p=128)  # [128, 32, 2048]
    wa_v = wa_t[:].rearrange("(ko p) m -> p ko m", p=128)  # [128, 32, 768]
    wp_v = wp_t[:].rearrange("(ko p) m -> p ko m", p=128)  # [128, 32, 512]
    ag_vs = [a[:].rearrange("(ko p) n -> p ko n", p=128) for a in ag_outs]

    with tile.TileContext(nc) as tc:
        # ---------- persistent pools ----------
        const_pool = tc.alloc_tile_pool(name="consts", bufs=1)
        qkv_pool = tc.alloc_tile_pool(name="qkv", bufs=1)
        vnat_pool = tc.alloc_tile_pool(name="vnat", bufs=1)
        wp_pool = tc.alloc_tile_pool(name="wp", bufs=1)

        ones_sb = const_pool.tile([128, 128], BF16)
        ident_sb = const_pool.tile([128, 128], BF16)
        rot_sb = const_pool.tile([128, HD], BF16)
        stair_sb = const_pool.tile([128, 512], BF16)

        qkv_sb = qkv_pool.tile([128, MT, S], BF16)  # 24KB/part
        v_nat = vnat_pool.tile([128, ST, HD], BF16)  # 4KB/part
        wp_sb = wp_pool.tile([128, KC, P_SHARD], BF16)  # 32KB/part

        # ---------- phase A: qkv_t = wa_shard @ hs.T, rope fused ----------
        with (
            tc.tile_pool(name="wa", bufs=1) as wa_pool,
            tc.tile_pool(name="hs", bufs=3) as hs_pool,
            tc.tile_pool(name="rope", bufs=1) as rope_pool,
            tc.tile_pool(name="rs", bufs=2) as rs_pool,
            tc.tile_pool(name="psA", bufs=1, space="PSUM") as psA,
            tc.tile_pool(name="psR", bufs=2, space="PSUM") as psR,
        ):
            wa_sb = wa_pool.tile([128, KC, M_SHARD], BF16)  # 48KB/part
            cos_sb = rope_pool.tile([128, S], BF16, tag="cos")
            sin_sb = rope_pool.tile([128, S], BF16, tag="sin")

            # wa chunks first on scalar (PE needs chunk0 immediately); the
            # first hs slab is first on sync.  Constants / rope tables /
            # w_proj stream behind — they're needed much later.
            for k0, k1 in ((0, 4), (4, 8), (8, 16), (16, 24), (24, 32)):
                nc.scalar.dma_start(
                    out=wa_sb[:, k0:k1, :], in_=wa_v[:, k0:k1, :]
                )
            nc.scalar.dma_start(out=rot_sb, in_=rot_t[:])
            nc.scalar.dma_start(out=ones_sb, in_=ones_in[:])
            nc.scalar.dma_start(out=ident_sb, in_=ident_in[:])
            nc.scalar.dma_start(out=stair_sb, in_=stair_in[:])
            for kk in range(0, KC, 8):
                nc.scalar.dma_start(
                    out=wp_sb[:, kk : kk + 8, :], in_=wp_v[:, kk : kk + 8, :]
                )

            # slab list; nb0 starts with two small slabs so the PE can begin
            # within ~2us of kernel start
            slabs = [(0, 0, 4), (0, 4, 8), (0, 8, 16), (0, 16, 24), (0, 24, 32)]
            for nb in range(1, NB):
                slabs += [(nb, ks, ks + 8) for ks in range(0, KC, 8)]
            slab_tiles: dict[int, bass.AP] = {}

            def issue_slab(i):
                nb, k0, k1 = slabs[i]
                t = hs_pool.tile([128, 8, 512], BF16, name="hs_slab")
                nc.sync.dma_start(
                    out=t[:, 0 : k1 - k0, :],
                    in_=hs_v[:, k0:k1, nb * 512 : (nb + 1) * 512],
                )
                slab_tiles[i] = t

            issue_slab(0)
            issue_slab(1)
            nc.sync.dma_start(out=cos_sb, in_=cos_t[:])
            nc.sync.dma_start(out=sin_sb, in_=sin_t[:])

            psums = None
            for i, (nb, k0, k1) in enumerate(slabs):
                blk = slice(nb * 512, (nb + 1) * 512)
                if k0 == 0:
                    psums = [
                        psA.tile([128, 512], F32, tag=f"ps{m}", name=f"psA{m}")
                        for m in range(MT)
                    ]
                hs_slab = slab_tiles.pop(i)
                for k in range(k0, k1):
                    for m in range(MT):
                        nc.tensor.matmul(
                            psums[m],
                            lhsT=wa_sb[:, k, m * 128 : (m + 1) * 128],
                            rhs=hs_slab[:, k - k0, :],
                            start=(k == 0),
                            stop=(k == KC - 1),
                        )
                # keep two slabs in flight ahead of the consumer so the
                # per-nb copies/transposes/rope never delay the stream
                if i + 2 < len(slabs):
                    issue_slab(i + 2)
                if k1 != KC:
                    continue
                for m in range(MT):
                    nc.vector.tensor_copy(out=qkv_sb[:, m, blk], in_=psums[m])

                # v natural layout via DMA-transpose (sync queue, tiny)
                for u in range(4):
                    nc.sync.dma_start_transpose(
                        v_nat[:, nb * 4 + u, :],
                        qkv_sb[:, GROUP + 1, nb * 512 + u * 128 : nb * 512 + (u + 1) * 128],
                    )

                # rope this seq block, k tile first (attention needs kT whole)
                for t in (GROUP, 0, 1, 2, 3):
                    x = qkv_sb[:, t, blk]
                    rp = psR.tile([128, 512], F32, name="rp")
                    nc.tensor.matmul(rp, lhsT=rot_sb[:], rhs=x, start=True, stop=True)
                    rs = rs_pool.tile([128, 512], BF16, name="rs")
                    nc.vector.tensor_mul(rs, rp, sin_sb[:, blk])
                    nc.vector.tensor_mul(x, x, cos_sb[:, blk])
                    nc.vector.tensor_add(x, x, rs)

        # ---------- phase B: attention + chunked AG + c_proj ----------
        with (
            tc.tile_pool(name="pt", bufs=1) as pt_pool,
            tc.tile_pool(name="attn", bufs=2) as attn_pool,
            tc.tile_pool(name="lh", bufs=4) as lh_pool,
            tc.tile_pool(name="ysb", bufs=2) as y_pool,
            tc.tile_pool(name="psS", bufs=3, space="PSUM") as psS,
            tc.tile_pool(name="psL", bufs=1, space="PSUM") as psL,
            tc.tile_pool(name="psO", bufs=2, space="PSUM") as psO,
            tc.tile_pool(name="psC", bufs=2, space="PSUM") as psC,
        ):
            # pre-zero the shifted-diagonal pt tags' dead zones once
            for r in (128, 256, 384):
                ptd = pt_pool.tile(
                    [128, 512], BF16, tag=f"ptd{r}", name=f"ptd{r}"
                )
                nc.vector.memset(ptd[:, 0:r], 0.0)

            kT = qkv_sb[:, GROUP, :]
            lh_tiles = {}

            def emit_lh(iq):
                # gpsimd SWDGE: the only other gpsimd work is the collective
                # triggers, whose waits resolve in the same order, so the
                # scheduler can't park an AllGather-blocked DMA in front of
                # anything latency-critical (it did exactly that on the
                # scalar queue: 29us exp stall; on sync it stalled the CC
                # stream itself: 96us).
                tiles = []
                for sub in range(4):
                    lh = lh_pool.tile([128, KC, 128], BF16, tag="lh", name="lh")
                    nc.gpsimd.dma_start(
                        out=lh, in_=ag_vs[iq][:, :, sub * 128 : (sub + 1) * 128]
                    )
                    tiles.append(lh)
                return tiles

            def cproj(iq, tiles):
                for sub in range(4):
                    lh = tiles[sub]
                    yp = psC.tile([128, 512], F32, name="yp")
                    for k in range(KC):
                        nc.tensor.matmul(
                            yp,
                            lhsT=lh[:, k, :],
                            rhs=wp_sb[:, k, :],
                            start=(k == 0),
                            stop=(k == KC - 1),
                        )
                    ysb = y_pool.tile([128, P_SHARD], BF16, name="ysb")
                    nc.vector.tensor_copy(out=ysb, in_=yp)
                    nc.sync.dma_start(
                        out=y_out[(iq * 4 + sub) * 128 : (iq * 4 + sub + 1) * 128, :],
                        in_=ysb,
                    )

            order = [3, 2, 1, 0]
            for idx, iq in enumerate(order):
                njb = 4 * iq + 4
                q0 = iq * 512
                for h in range(GROUP):
                    l_ps = psL.tile([128, 512], F32, name="l_ps")
                    o_ps = psO.tile([128, 512], F32, name="o_ps")

                    # round r: key tile j=r; last 4 rounds are diagonal with
                    # shifted q-range [rel, 512) and a staircase bias matmul
                    sts = []  # per round: (st psum, pt tile, rel)

                    def emit_st(r):
                        j = r
                        rel = max(0, (j - 4 * iq) * 128)
                        st = psS.tile([128, 512], F32, name="st")
                        nc.tensor.matmul(
                            st[:, rel:512],
                            lhsT=kT[:, j * 128 : (j + 1) * 128],
                            rhs=qkv_sb[:, h, q0 + rel : q0 + 512],
                            start=True,
                            stop=True,
                        )
                        if rel > 0:
                            # triangle bias only lives in the first 128 cols
                            # of the slice; stop already satisfied by the st
                            # matmul, so skip group bookkeeping
                            nc.tensor.matmul(
                                st[:, rel : rel + 128],
                                lhsT=ident_sb[:],
                                rhs=stair_sb[:, 0:128],
                                start=False,
                                stop=False,
                                skip_group_check=True,
                            )
                        tag = f"ptd{rel}" if rel else "pt"
                        bufs = 1 if rel else 4
                        pt = pt_pool.tile(
                            [128, 512], BF16, tag=tag, bufs=bufs, name="pt"
                        )
                        nc.scalar.activation(
                            out=pt[:, rel:512],
                            in_=st[:, rel:512],
                            func=mybir.ActivationFunctionType.Exp,
                            scale=SCALE,
                        )
                        sts.append((st, pt, rel))

                    def emit_st_diag0(r):
                        # diagonal tile at rel==0 (j == 4*iq): triangle bias
                        j = r
                        st = psS.tile([128, 512], F32, name="st")
                        nc.tensor.matmul(
                            st,
                            lhsT=kT[:, j * 128 : (j + 1) * 128],
                            rhs=qkv_sb[:, h, q0 : q0 + 512],
                            start=True,
                            stop=True,
                        )
                        nc.tensor.matmul(
                            st[:, 0:128],
                            lhsT=ident_sb[:],
                            rhs=stair_sb[:, 0:128],
                            start=False,
                            stop=False,
                            skip_group_check=True,
                        )
                        pt = pt_pool.tile([128, 512], BF16, tag="pt", bufs=4, name="pt")
                        nc.scalar.activation(
                            out=pt,
                            in_=st,
                            func=mybir.ActivationFunctionType.Exp,
                            scale=SCALE,
                        )
                        sts.append((st, pt, 0))

                    def emit_round_st(r):
                        if r == 4 * iq:
                            emit_st_diag0(r)
                        else:
                            emit_st(r)

                    def emit_lo(r):
                        _, pt, rel = sts[r]
                        j = r
                        # only columns [rel, 512) are live for this key tile
                        # (round 0 is always full width, so every column gets
                        # its psum start=reset there); stop flags are partial
                        # so skip group bookkeeping and rely on subtile deps
                        nc.tensor.matmul(
                            l_ps[:, rel:512],
                            lhsT=ones_sb[:],
                            rhs=pt[:, rel:512],
                            start=(r == 0),
                            stop=(r == njb - 1),
                            skip_group_check=True,
                        )
                        nc.tensor.matmul(
                            o_ps[:, rel:512],
                            lhsT=v_nat[:, j, :],
                            rhs=pt[:, rel:512],
                            start=(r == 0),
                            stop=(r == njb - 1),
                            skip_group_check=True,
                        )

                    depth = min(3, njb)
                    for r in range(depth):
                        emit_round_st(r)
                    for r in range(njb):
                        emit_lo(r)
                        if r + depth < njb:
                            emit_round_st(r + depth)

                    linv = attn_pool.tile([128, 512], F32, tag="linv", name="linv")
                    nc.vector.reciprocal_approx_fast(out=linv, in_=l_ps)
                    at = attn_pool.tile([128, 512], BF16, tag="at", name="at")
                    nc.vector.tensor_mul(at, o_ps, linv)
                    nc.sync.dma_start(
                        out=ag_ins[iq][h * 128 : (h + 1) * 128, :], in_=at
                    )

                # lh loads for the chunk whose AG completed two chunks ago,
                # emitted BEFORE this chunk's AG trigger: the gpsimd engine
                # executes in order, so lh desc-gen must not sit behind an
                # AG trigger that waits on not-yet-computed at stores
                if idx >= 2:
                    lh_tiles[order[idx - 2]] = emit_lh(order[idx - 2])

                # seq-chunked AllGather; overlaps the remaining compute
                nc.gpsimd.collective_compute(
                    "AllGather",
                    mybir.AluOpType.bypass,
                    replica_groups=[list(range(NCORES))],
                    ins=[ag_ins[iq][:]],
                    outs=[ag_outs[iq][:]],
                )
            for iq in (order[-2], order[-1]):
                lh_tiles[iq] = emit_lh(iq)

            # c_projs after all attention, pinned late in the scheduler's
            # model time (tile_wait_until) so it cannot hoist them ahead of
            # attention work: their lh loads wait on AllGathers whose ~45us
            # trigger-to-done latency the scheduler does not model (measured
            # 26-29us PE stalls from exactly that hoist)
            t_cproj = {3: 0.38, 2: 0.42, 1: 0.46, 0: 0.50}
            for iq in order:
                with tc.tile_wait_until(t_cproj[iq]):
                    cproj(iq, lh_tiles.pop(iq))

        wp_pool.release()
        vnat_pool.release()
        qkv_pool.release()
        const_pool.release()

    nc.compile()
    return nc


_CACHED = {}


def _get_module():
    if "nc" not in _CACHED:
        _CACHED["nc"] = build_module()
    return _CACHED["nc"]


def make_in_maps(hidden_states, w_attn, w_proj, rope_cos, rope_sin):
    bf = ml_dtypes.bfloat16
    hidden_states = np.asarray(hidden_states, dtype=np.float32)
    w_attn = np.asarray(w_attn, dtype=np.float32)
    w_proj = np.asarray(w_proj, dtype=np.float32)
    rope_cos = np.asarray(rope_cos, dtype=np.float32)
    rope_sin = np.asarray(rope_sin, dtype=np.float32)

    hs_t = np.ascontiguousarray(hidden_states.reshape(S, H).T).astype(bf)
    cos_t = np.ascontiguousarray(rope_cos.T).astype(bf)
    sin_t = np.ascontiguousarray(rope_sin.T).astype(bf)

    # rotate-half as a matmul: rot(x) = R @ x for x in [HD, S] layout,
    # rot_t = R.T so that lhsT.T @ x = R @ x
    rot_t = np.zeros((HD, HD), dtype=np.float32)
    half = HD // 2
    rot_t[half + np.arange(half), np.arange(half)] = -1.0
    rot_t[np.arange(half), half + np.arange(half)] = 1.0

    # causal triangle bias: stair[k, u] = NEG iff u < k (else 0); a diag
    # key tile at relative offset rel uses cols [0, 512-rel)
    kk, uu = np.meshgrid(np.arange(128), np.arange(512), indexing="ij")
    stair = np.where(uu < kk, NEG, 0.0).astype(np.float32)

    ones = np.ones((128, 128), dtype=np.float32)
    ident = np.eye(128, dtype=np.float32)

    in_maps = []
    for i in range(NCORES):
        wa_sh = w_attn[i * M_SHARD : (i + 1) * M_SHARD, :]
        wp_sh = w_proj[i * P_SHARD : (i + 1) * P_SHARD, :]
        in_maps.append(
            {
                "hs_t": hs_t,
                "wa_t": np.ascontiguousarray(wa_sh.T).astype(bf),
                "wp_t": np.ascontiguousarray(wp_sh.T).astype(bf),
                "cos_t": cos_t,
                "sin_t": sin_t,
                "rot_t": rot_t.astype(bf),
                "stair_in": stair.astype(bf),
                "ones_in": ones.astype(bf),
                "ident_in": ident.astype(bf),
            }
        )
    return in_maps


def kernel(hidden_states, w_attn, w_proj, rope_cos, rope_sin, **_unused):
    nc = _get_module()
    in_maps = make_in_maps(hidden_states, w_attn, w_proj, rope_cos, rope_sin)
    res = run_bass_kernel_spmd(nc, in_maps, core_ids=list(range(NCORES)))

    out = np.empty((S, H), dtype=np.float32)
    for i in range(NCORES):
        out[:, i * P_SHARD : (i + 1) * P_SHARD] = res.results[i]["y"].astype(
            np.float32
        )
    return out.reshape(B, S, H)


# revision 26
# speedup vs baseline: 1.2739x; 1.0135x over previous
"""Trainium2 Bass kernel for fused causal GQA attention block.

Reference computation (B=1, S=2048, H=4096, NH=32, NKV=8, HD=128):
    qkv = hs @ w_attn.T; rope(q), rope(k); causal GQA attention;
    out @ w_proj.T

Sharding (8 cores, tensor parallel): core i owns kv-group i = rows
[i*768, (i+1)*768) of w_attn (4 q heads + 1 k + 1 v head) and rows
[i*512, (i+1)*512) of w_proj.  Each core computes its 4 heads of
attention output transposed (feature-major); a seq-chunked AllGather
assembles attnT on every core, and each core computes its 512 output
columns of the final projection per seq chunk.

v2 design (vs the f32r baseline):
  * all matmul/DMA data in bf16 (host-converted) -- halves HBM and
    collective traffic and the LDWEIGHTS stream; psums stay fp32.
  * RoPE + v-transposes interleaved into phase A per 512-seq block so
    attention starts immediately after the last qkv block.
  * attention runs seq chunks in order iq=3,2,1,0 (largest first) with
    each chunk's AllGather issued right away and its c_proj placed one
    chunk later, so every collective hides under compute.
  * causal mask applied on the PE as a staircase bias-matmul into the
    scores psum (lhsT=identity) -- no DVE hop between exp and l/o.
  * scores software-pipelined 3 deep over key tiles; exp on ACT.
  * softmax reciprocal via reciprocal_approx_fast (~5x faster).
"""

import sys

sys.path.insert(0, "/opt/trn_rl_repo")

import ml_dtypes
import numpy as np

import concourse.bass as bass
import concourse.tile as tile
from concourse import bacc, mybir
from concourse.bass_utils import run_bass_kernel_spmd

F32 = mybir.dt.float32
BF16 = mybir.dt.bfloat16

B, S, H = 1, 2048, 4096
NH, NKV, HD = 32, 8, 128
GROUP = NH // NKV  # 4
SCALE = 0.08838834764831845
NCORES = 8

M_SHARD = (GROUP + 2) * HD  # 768 rows of w_attn per core
P_SHARD = H // NCORES  # 512 rows of w_proj per core

KC = H // 128  # 32 contraction chunks of the model dim
NB = S // 512  # 4 seq blocks of 512
MT = M_SHARD // 128  # 6 row tiles of qkv_t
ST = S // 128  # 16 seq tiles of 128
NEG = -1.0e9


def build_module() -> bass.Bass:
    nc = bacc.Bacc(
        "TRN2",
        target_bir_lowering=False,
        debug=False,
        num_devices=NCORES,
    )

    hs_t = nc.dram_tensor("hs_t", [H, S], BF16, kind="ExternalInput")
    wa_t = nc.dram_tensor("wa_t", [H, M_SHARD], BF16, kind="ExternalInput")
    wp_t = nc.dram_tensor("wp_t", [H, P_SHARD], BF16, kind="ExternalInput")
    cos_t = nc.dram_tensor("cos_t", [HD, S], BF16, kind="ExternalInput")
    sin_t = nc.dram_tensor("sin_t", [HD, S], BF16, kind="ExternalInput")
    rot_t = nc.dram_tensor("rot_t", [HD, HD], BF16, kind="ExternalInput")
    stair_in = nc.dram_tensor("stair_in", [128, 512], BF16, kind="ExternalInput")
    ones_in = nc.dram_tensor("ones_in", [128, 128], BF16, kind="ExternalInput")
    ones32_in = nc.dram_tensor("ones32_in", [128, 128], mybir.dt.float32r, kind="ExternalInput")
    ident_in = nc.dram_tensor("ident_in", [128, 128], BF16, kind="ExternalInput")
    y_out = nc.dram_tensor("y", [S, P_SHARD], BF16, kind="ExternalOutput")

    # per-seq-chunk collective buffers
    ag_ins = [
        nc.dram_tensor(f"ag_in{i}", [GROUP * HD, 512], BF16, kind="Internal")
        for i in range(NB)
    ]
    ag_outs = [
        nc.dram_tensor(
            f"ag_out{i}", [H, 512], BF16, kind="Internal", addr_space="Shared"
        )
        for i in range(NB)
    ]

    # DRAM views with 128-partition tiling of the contraction axis
    hs_v = hs_t[:].rearrange("(ko p) n -> p ko n", p=128)  # [128, 32, 2048]
    wa_v = wa_t[:].rearrange("(ko p) m -> p ko m", p=128)  # [128, 32, 768]
    wp_v = wp_t[:].rearrange("(ko p) m -> p ko m", p=128)  # [128, 32, 512]
    ag_vs = [a[:].rearrange("(ko p) n -> p ko n", p=128) for a in ag_outs]

    with tile.TileContext(nc) as tc:
        # ---------- persistent pools ----------
        const_pool = tc.alloc_tile_pool(name="consts", bufs=1)
        qkv_pool = tc.alloc_tile_pool(name="qkv", bufs=1)
        vnat_pool = tc.alloc_tile_pool(name="vnat", bufs=1)
        wp_pool = tc.alloc_tile_pool(name="wp", bufs=1)

        ones_sb = const_pool.tile([128, 128], BF16)
        ones32_sb = const_pool.tile([128, 128], mybir.dt.float32r)
        ident_sb = const_pool.tile([128, 128], BF16)
        rot_sb = const_pool.tile([128, HD], BF16)
        stair_sb = const_pool.tile([128, 512], BF16)

        qkv_sb = qkv_pool.tile([128, MT, S], BF16)  # 24KB/part
        v_nat = vnat_pool.tile([128, ST, HD], BF16)  # 4KB/part
        wp_sb = wp_pool.tile([128, KC, P_SHARD], BF16)  # 32KB/part

        # ---------- phase A: qkv_t = wa_shard @ hs.T, rope fused ----------
        with (
            tc.tile_pool(name="wa", bufs=1) as wa_pool,
            tc.tile_pool(name="hs", bufs=3) as hs_pool,
            tc.tile_pool(name="rope", bufs=1) as rope_pool,
            tc.tile_pool(name="rs", bufs=2) as rs_pool,
            tc.tile_pool(name="psA", bufs=1, space="PSUM") as psA,
            tc.tile_pool(name="psR", bufs=2, space="PSUM") as psR,
        ):
            wa_sb = wa_pool.tile([128, KC, M_SHARD], BF16)  # 48KB/part
            cos_sb = rope_pool.tile([128, S], BF16, tag="cos")
            sin_sb = rope_pool.tile([128, S], BF16, tag="sin")

            # wa chunks first on scalar (PE needs chunk0 immediately); the
            # first hs slab is first on sync.  Constants / rope tables /
            # w_proj stream behind — they're needed much later.
            for k0, k1 in ((0, 4), (4, 8), (8, 16), (16, 24), (24, 32)):
                nc.scalar.dma_start(
                    out=wa_sb[:, k0:k1, :], in_=wa_v[:, k0:k1, :]
                )
            nc.scalar.dma_start(out=rot_sb, in_=rot_t[:])
            nc.scalar.dma_start(out=ones_sb, in_=ones_in[:])
            nc.scalar.dma_start(out=ones32_sb, in_=ones32_in[:])
            nc.scalar.dma_start(out=ident_sb, in_=ident_in[:])
            nc.scalar.dma_start(out=stair_sb, in_=stair_in[:])
            for kk in range(0, KC, 8):
                nc.scalar.dma_start(
                    out=wp_sb[:, kk : kk + 8, :], in_=wp_v[:, kk : kk + 8, :]
                )

            # slab list; nb0 starts with two small slabs so the PE can begin
            # within ~2us of kernel start
            slabs = [(0, 0, 4), (0, 4, 8), (0, 8, 16), (0, 16, 24), (0, 24, 32)]
            for nb in range(1, NB):
                slabs += [(nb, ks, ks + 8) for ks in range(0, KC, 8)]
            slab_tiles: dict[int, bass.AP] = {}

            def issue_slab(i):
                nb, k0, k1 = slabs[i]
                t = hs_pool.tile([128, 8, 512], BF16, name="hs_slab")
                nc.sync.dma_start(
                    out=t[:, 0 : k1 - k0, :],
                    in_=hs_v[:, k0:k1, nb * 512 : (nb + 1) * 512],
                )
                slab_tiles[i] = t

            issue_slab(0)
            issue_slab(1)
            nc.sync.dma_start(out=cos_sb, in_=cos_t[:])
            nc.sync.dma_start(out=sin_sb, in_=sin_t[:])

            psums = None
            for i, (nb, k0, k1) in enumerate(slabs):
                blk = slice(nb * 512, (nb + 1) * 512)
                if k0 == 0:
                    psums = [
                        psA.tile([128, 512], F32, tag=f"ps{m}", name=f"psA{m}")
                        for m in range(MT)
                    ]
                hs_slab = slab_tiles.pop(i)
                for k in range(k0, k1):
                    for m in range(MT):
                        nc.tensor.matmul(
                            psums[m],
                            lhsT=wa_sb[:, k, m * 128 : (m + 1) * 128],
                            rhs=hs_slab[:, k - k0, :],
                            start=(k == 0),
                            stop=(k == KC - 1),
                        )
                # keep two slabs in flight ahead of the consumer so the
                # per-nb copies/transposes/rope never delay the stream
                if i + 2 < len(slabs):
                    issue_slab(i + 2)
                if k1 != KC:
                    continue
                for m in range(MT):
                    nc.vector.tensor_copy(out=qkv_sb[:, m, blk], in_=psums[m])

                # v natural layout via DMA-transpose (sync queue, tiny)
                for u in range(4):
                    nc.sync.dma_start_transpose(
                        v_nat[:, nb * 4 + u, :],
                        qkv_sb[:, GROUP + 1, nb * 512 + u * 128 : nb * 512 + (u + 1) * 128],
                    )

                # rope this seq block, k tile first (attention needs kT whole)
                for t in (GROUP, 0, 1, 2, 3):
                    x = qkv_sb[:, t, blk]
                    rp = psR.tile([128, 512], F32, name="rp")
                    nc.tensor.matmul(rp, lhsT=rot_sb[:], rhs=x, start=True, stop=True)
                    rs = rs_pool.tile([128, 512], BF16, name="rs")
                    nc.vector.tensor_mul(rs, rp, sin_sb[:, blk])
                    nc.vector.tensor_mul(x, x, cos_sb[:, blk])
                    nc.vector.tensor_add(x, x, rs)

        # ---------- phase B: attention + chunked AG + c_proj ----------
        with (
            tc.tile_pool(name="pt", bufs=1) as pt_pool,
            tc.tile_pool(name="attn", bufs=2) as attn_pool,
            tc.tile_pool(name="acc", bufs=2) as acc_pool,
            tc.tile_pool(name="lh", bufs=4) as lh_pool,
            tc.tile_pool(name="ysb", bufs=2) as y_pool,
            tc.tile_pool(name="psS", bufs=3, space="PSUM") as psS,
            tc.tile_pool(name="psL", bufs=1, space="PSUM") as psL,
            tc.tile_pool(name="psO", bufs=2, space="PSUM") as psO,
            tc.tile_pool(name="psC", bufs=2, space="PSUM") as psC,
        ):
            # pre-zero the shifted-diagonal pt tags' dead zones once
            for r in (128, 256, 384):
                ptd = pt_pool.tile(
                    [128, 512], BF16, tag=f"ptd{r}", name=f"ptd{r}"
                )
                nc.vector.memset(ptd[:, 0:r], 0.0)

            kT = qkv_sb[:, GROUP, :]
            lh_tiles = {}

            def emit_lh(iq):
                # gpsimd SWDGE: the only other gpsimd work is the collective
                # triggers, whose waits resolve in the same order, so the
                # scheduler can't park an AllGather-blocked DMA in front of
                # anything latency-critical (it did exactly that on the
                # scalar queue: 29us exp stall; on sync it stalled the CC
                # stream itself: 96us).
                tiles = []
                for sub in range(4):
                    lh = lh_pool.tile([128, KC, 128], BF16, tag="lh", name="lh")
                    nc.gpsimd.dma_start(
                        out=lh, in_=ag_vs[iq][:, :, sub * 128 : (sub + 1) * 128]
                    )
                    tiles.append(lh)
                return tiles

            def cproj(iq, tiles):
                for sub in range(4):
                    lh = tiles[sub]
                    yp = psC.tile([128, 512], F32, name="yp")
                    for k in range(KC):
                        nc.tensor.matmul(
                            yp,
                            lhsT=lh[:, k, :],
                            rhs=wp_sb[:, k, :],
                            start=(k == 0),
                            stop=(k == KC - 1),
                        )
                    ysb = y_pool.tile([128, P_SHARD], BF16, name="ysb")
                    nc.vector.tensor_copy(out=ysb, in_=yp)
                    nc.sync.dma_start(
                        out=y_out[(iq * 4 + sub) * 128 : (iq * 4 + sub + 1) * 128, :],
                        in_=ysb,
                    )

            order = [3, 2, 1, 0]
            for idx, iq in enumerate(order):
                njb = 4 * iq + 4
                q0 = iq * 512
                for h in range(GROUP):
                    l_ps = psL.tile([128, 512], F32, name="l_ps")
                    o_ps = psO.tile([128, 512], F32, name="o_ps")
                    acc = (
                        acc_pool.tile(
                            [128, 512], mybir.dt.float32r, tag="acc", name="acc"
                        )
                        if iq > 0
                        else None
                    )

                    # round r: key tile j=r; last 4 rounds are diagonal with
                    # shifted q-range [rel, 512) and a staircase bias matmul
                    sts = []  # per round: (st psum, pt tile, rel)

                    def emit_st(r):
                        j = r
                        rel = max(0, (j - 4 * iq) * 128)
                        st = psS.tile([128, 512], F32, name="st")
                        nc.tensor.matmul(
                            st[:, rel:512],
                            lhsT=kT[:, j * 128 : (j + 1) * 128],
                            rhs=qkv_sb[:, h, q0 + rel : q0 + 512],
                            start=True,
                            stop=True,
                        )
                        if rel > 0:
                            # triangle bias only lives in the first 128 cols
                            # of the slice; stop already satisfied by the st
                            # matmul, so skip group bookkeeping
                            nc.tensor.matmul(
                                st[:, rel : rel + 128],
                                lhsT=ident_sb[:],
                                rhs=stair_sb[:, 0:128],
                                start=False,
                                stop=False,
                                skip_group_check=True,
                            )
                        tag = f"ptd{rel}" if rel else "pt"
                        bufs = 1 if rel else 6
                        pt = pt_pool.tile(
                            [128, 512], BF16, tag=tag, bufs=bufs, name="pt"
                        )
                        nc.scalar.activation(
                            out=pt[:, rel:512],
                            in_=st[:, rel:512],
                            func=mybir.ActivationFunctionType.Exp,
                            scale=SCALE,
                        )
                        sts.append((st, pt, rel))

                    def emit_st_diag0(r):
                        # diagonal tile at rel==0 (j == 4*iq): triangle bias
                        j = r
                        st = psS.tile([128, 512], F32, name="st")
                        nc.tensor.matmul(
                            st,
                            lhsT=kT[:, j * 128 : (j + 1) * 128],
                            rhs=qkv_sb[:, h, q0 : q0 + 512],
                            start=True,
                            stop=True,
                        )
                        nc.tensor.matmul(
                            st[:, 0:128],
                            lhsT=ident_sb[:],
                            rhs=stair_sb[:, 0:128],
                            start=False,
                            stop=False,
                            skip_group_check=True,
                        )
                        pt = pt_pool.tile([128, 512], BF16, tag="pt", bufs=6, name="pt")
                        nc.scalar.activation(
                            out=pt,
                            in_=st,
                            func=mybir.ActivationFunctionType.Exp,
                            scale=SCALE,
                        )
                        sts.append((st, pt, 0))

                    def emit_round_st(r):
                        if r == 4 * iq:
                            emit_st_diag0(r)
                        else:
                            emit_st(r)

                    def emit_lo(r):
                        _, pt, rel = sts[r]
                        j = r
                        # off-diagonal rounds: row-sum contribution goes into
                        # an SBUF accumulator on DVE (folded into l_ps by one
                        # f32r ones-matmul in the epilogue); diagonal rounds
                        # keep the narrowed ones-matmul.  Round 4*iq (diag0,
                        # full width) carries start=True for the whole psum.
                        if r < 4 * iq:
                            if r == 0:
                                nc.vector.tensor_copy(out=acc, in_=pt[:])
                            else:
                                nc.vector.tensor_add(acc, acc, pt[:])
                        else:
                            nc.tensor.matmul(
                                l_ps[:, rel:512],
                                lhsT=ones_sb[:],
                                rhs=pt[:, rel:512],
                                start=(r == 4 * iq),
                                stop=(iq == 0 and r == njb - 1),
                                skip_group_check=True,
                            )
                        nc.tensor.matmul(
                            o_ps[:, rel:512],
                            lhsT=v_nat[:, j, :],
                            rhs=pt[:, rel:512],
                            start=(r == 0),
                            stop=(r == njb - 1),
                            skip_group_check=True,
                        )

                    depth = min(3, njb)
                    for r in range(depth):
                        emit_round_st(r)
                    for r in range(njb):
                        emit_lo(r)
                        if r + depth < njb:
                            emit_round_st(r + depth)

                    if iq > 0:
                        nc.tensor.matmul(
                            l_ps,
                            lhsT=ones32_sb[:],
                            rhs=acc,
                            start=False,
                            stop=True,
                            skip_group_check=True,
                        )
                    linv = attn_pool.tile([128, 512], F32, tag="linv", name="linv")
                    nc.vector.reciprocal_approx_fast(out=linv, in_=l_ps)
                    at = attn_pool.tile([128, 512], BF16, tag="at", name="at")
                    nc.vector.tensor_mul(at, o_ps, linv)
                    nc.sync.dma_start(
                        out=ag_ins[iq][h * 128 : (h + 1) * 128, :], in_=at
                    )

                # lh loads for the chunk whose AG completed two chunks ago,
                # emitted BEFORE this chunk's AG trigger: the gpsimd engine
                # executes in order, so lh desc-gen must not sit behind an
                # AG trigger that waits on not-yet-computed at stores
                if idx >= 2:
                    lh_tiles[order[idx - 2]] = emit_lh(order[idx - 2])

                # seq-chunked AllGather; overlaps the remaining compute
                nc.gpsimd.collective_compute(
                    "AllGather",
                    mybir.AluOpType.bypass,
                    replica_groups=[list(range(NCORES))],
                    ins=[ag_ins[iq][:]],
                    outs=[ag_outs[iq][:]],
                )
            for iq in (order[-2], order[-1]):
                lh_tiles[iq] = emit_lh(iq)

            # c_projs after all attention, pinned late in the scheduler's
            # model time (tile_wait_until) so it cannot hoist them ahead of
            # attention work: their lh loads wait on AllGathers whose ~45us
            # trigger-to-done latency the scheduler does not model (measured
            # 26-29us PE stalls from exactly that hoist)
            t_cproj = {3: 0.38, 2: 0.42, 1: 0.46, 0: 0.50}
            for iq in order:
                with tc.tile_wait_until(t_cproj[iq]):
                    cproj(iq, lh_tiles.pop(iq))

        wp_pool.release()
        vnat_pool.release()
        qkv_pool.release()
        const_pool.release()

    nc.compile()
    return nc


_CACHED = {}


def _get_module():
    if "nc" not in _CACHED:
        _CACHED["nc"] = build_module()
    return _CACHED["nc"]


def make_in_maps(hidden_states, w_attn, w_proj, rope_cos, rope_sin):
    bf = ml_dtypes.bfloat16
    hidden_states = np.asarray(hidden_states, dtype=np.float32)
    w_attn = np.asarray(w_attn, dtype=np.float32)
    w_proj = np.asarray(w_proj, dtype=np.float32)
    rope_cos = np.asarray(rope_cos, dtype=np.float32)
    rope_sin = np.asarray(rope_sin, dtype=np.float32)

    hs_t = np.ascontiguousarray(hidden_states.reshape(S, H).T).astype(bf)
    cos_t = np.ascontiguousarray(rope_cos.T).astype(bf)
    sin_t = np.ascontiguousarray(rope_sin.T).astype(bf)

    # rotate-half as a matmul: rot(x) = R @ x for x in [HD, S] layout,
    # rot_t = R.T so that lhsT.T @ x = R @ x
    rot_t = np.zeros((HD, HD), dtype=np.float32)
    half = HD // 2
    rot_t[half + np.arange(half), np.arange(half)] = -1.0
    rot_t[np.arange(half), half + np.arange(half)] = 1.0

    # causal triangle bias: stair[k, u] = NEG iff u < k (else 0); a diag
    # key tile at relative offset rel uses cols [0, 512-rel)
    kk, uu = np.meshgrid(np.arange(128), np.arange(512), indexing="ij")
    stair = np.where(uu < kk, NEG, 0.0).astype(np.float32)

    ones = np.ones((128, 128), dtype=np.float32)
    ident = np.eye(128, dtype=np.float32)

    in_maps = []
    for i in range(NCORES):
        wa_sh = w_attn[i * M_SHARD : (i + 1) * M_SHARD, :]
        wp_sh = w_proj[i * P_SHARD : (i + 1) * P_SHARD, :]
        in_maps.append(
            {
                "hs_t": hs_t,
                "wa_t": np.ascontiguousarray(wa_sh.T).astype(bf),
                "wp_t": np.ascontiguousarray(wp_sh.T).astype(bf),
                "cos_t": cos_t,
                "sin_t": sin_t,
                "rot_t": rot_t.astype(bf),
                "stair_in": stair.astype(bf),
                "ones_in": ones.astype(bf),
                "ones32_in": ones,
                "ident_in": ident.astype(bf),
            }
        )
    return in_maps


def kernel(hidden_states, w_attn, w_proj, rope_cos, rope_sin, **_unused):
    nc = _get_module()
    in_maps = make_in_maps(hidden_states, w_attn, w_proj, rope_cos, rope_sin)
    res = run_bass_kernel_spmd(nc, in_maps, core_ids=list(range(NCORES)))

    out = np.empty((S, H), dtype=np.float32)
    for i in range(NCORES):
        out[:, i * P_SHARD : (i + 1) * P_SHARD] = res.results[i]["y"].astype(
            np.float32
        )
    return out.reshape(B, S, H)
